# revision 22
# baseline (speedup 1.0000x reference)
"""Trainium2 Bass kernel for nn_MetaLearningCrisisMemory (retrieval_knn).

Self-contained: kernel(**inputs) -> np.ndarray [6154] fp32.

v2 strategy (8-way SPMD, memory-bound target):
 - Host-fold wk into K (kh = K @ wk.T) and wv into V (vh = V @ wv.T): the
   two big device passes become pure streamed sweeps. kh/vh shipped fp8
   (e4m3); output-norm analysis shows the attended section carries ~0.07%
   of output norm^2, so fp8 noise there is negligible.
 - Scores bounded (~|1.3|): exp without max-subtraction; softmax
   normalization Z rides along the u-AllReduce. No flash-max machinery.
 - Matmuls in vector-stationary orientation with N=512 moving columns:
   ~500 PE instructions total (vs 3225 in v1 at a fixed ~213ns each).
 - 5 AllReduces: enc, qh, u(+Z+top5 slots), attended, m2.
 - All small Linears tensor-parallel with bf16 host-pre-transposed shards.
"""

import numpy as np
import ml_dtypes

import concourse.bass as bass
import concourse.mybir as mybir
import concourse.tile as tile
from concourse import bacc, bass_utils
from concourse.bass import ts, ds
from concourse.masks import make_identity

f32 = mybir.dt.float32
bf16 = mybir.dt.bfloat16
f8 = mybir.dt.float8e4
AX = mybir.AxisListType
ALU = mybir.AluOpType
ACTF = mybir.ActivationFunctionType

NCN = 8
INPUT_DIM, HID, MEM, NPROTO = 4096, 2048, 50000, 64
H2 = HID // 2                  # 1024
NH = 8
DQ = H2 // NH                  # 128
DV = HID // NH                 # 256
TOPK = 5
EPS = 1e-5
MSH = MEM // NCN               # 6250 rows per core
MPAD = 6272                    # padded to 49 * 128
MT = MPAD // 128               # 49 m-tiles
NCH = 13                       # 12 chunks of 512 + 1 of 128
MVALID_TAIL = 106              # valid rows in tile 48 (6250 - 48*128)
OUT_N = 3 + 1 + 1 + TOPK + 3 * HID  # 6154
ISCALE = 1.0 / float(np.sqrt(np.float32(DQ)))
S8 = 32.0                      # fp8 pre-scale for the query


def _din(nc, name, shape, dt=f32):
    return nc.dram_tensor(name, list(shape), dt, kind="ExternalInput")


def build_nc(debug_taps=False):
    nc = bacc.Bacc("TRN2", target_bir_lowering=False, debug=False,
                   enable_asserts=False, num_devices=NCN)

    # ---- I/O ----
    obs = _din(nc, "obs", (128, 32), bf16)
    w1T = _din(nc, "w1T", (INPUT_DIM, 256), bf16)
    ce_b1r = _din(nc, "ce_b1r", (1, 256))
    bn1_sc = _din(nc, "bn1_sc", (1, 256))
    bn1_sh = _din(nc, "bn1_sh", (1, 256))
    ce_w2T = _din(nc, "ce_w2T", (256, HID), bf16)
    ce_b2 = _din(nc, "ce_b2", (128, 16))
    ce_b2r = _din(nc, "ce_b2r", (1, H2))
    qe_w1T = _din(nc, "qe_w1T", (HID, 256), bf16)
    qe_b1r = _din(nc, "qe_b1r", (1, 256))
    wq2T = _din(nc, "wq2T", (256, H2), bf16)
    qbias = _din(nc, "qbias", (128, 8))
    bk8 = _din(nc, "bk8", (128, 8))
    khT = _din(nc, "khT", (H2, MPAD), f8)
    vh = _din(nc, "vh", (MPAD, HID), f8)
    woT = _din(nc, "woT", (256, HID), bf16)
    bob = _din(nc, "bob", (128, 16))
    protos = _din(nc, "protos", (NPROTO, H2))
    mp1eT = _din(nc, "mp1eT", (HID, 256), bf16)
    mp1aT = _din(nc, "mp1aT", (HID, 256), bf16)
    mp1pT = _din(nc, "mp1pT", (H2, 256), bf16)
    mp_b1r = _din(nc, "mp_b1r", (1, 256))
    bn2_sc = _din(nc, "bn2_sc", (1, 256))
    bn2_sh = _din(nc, "bn2_sh", (1, 256))
    mp_w2T = _din(nc, "mp_w2T", (256, H2), bf16)
    mp_b2_8 = _din(nc, "mp_b2_8", (128, 8))
    mp_w3T = _din(nc, "mp_w3T", (H2, 4), bf16)
    mp_b3 = _din(nc, "mp_b3", (4, 1))
    oh8 = _din(nc, "oh8", (8, 1))
    slot_mask = _din(nc, "slot_mask", (1, 5 * NCN))
    out = nc.dram_tensor("out", [OUT_N], f32, kind="ExternalOutput")
    dbg = {}
    if debug_taps:
        for nm, shp in (("dbg_w0", [8, 512]), ("dbg_qh", [128, 8]),
                        ("dbg_u", [8, HID]), ("dbg_ctx", [128, 2]),
                        ("dbg_z", [8, 16]), ("dbg_m1", [1, 256]),
                        ("dbg_h", [1, 256]), ("dbg_t", [1, 256])):
            dbg[nm] = nc.dram_tensor(nm, shp, f32, kind="ExternalOutput")

    RG = [list(range(NCN))]

    with tile.TileContext(nc) as tc:
        import contextlib
        with contextlib.ExitStack() as stk:
            cpool = stk.enter_context(tc.tile_pool(name="cpool", bufs=1))
            vpool = stk.enter_context(tc.tile_pool(name="vpool", bufs=3))
            psx = stk.enter_context(tc.tile_pool(name="psx", bufs=1, space="PSUM"))
            pss = stk.enter_context(tc.tile_pool(name="pss", bufs=1, space="PSUM"))
            pstr = stk.enter_context(tc.tile_pool(name="pstr", bufs=2, space="PSUM"))
            psu = stk.enter_context(tc.tile_pool(name="psu", bufs=1, space="PSUM"))
            dpool = stk.enter_context(tc.tile_pool(name="dpool", bufs=1, space="DRAM"))

            def dma(dst, src):
                nc.sync.dma_start(out=dst, in_=src)

            def load(shape, dram_t, tag, dt=f32):
                t = cpool.tile(list(shape), dt, tag=tag)
                dma(t[:], dram_t.ap())
                return t

            # ---- CC warm-up first: pulls the CC entry barrier to t~0 and
            # absorbs the cold-stream cost under the front compute
            d_w_i = dpool.tile([1, 8], f32, tag="d_w_i")
            d_w_o = dpool.tile([1, 8], f32, tag="d_w_o")
            nc.gpsimd.collective_compute("AllReduce", ALU.add, replica_groups=RG,
                                         ins=[d_w_i.opt()], outs=[d_w_o.opt()])

            # ---- constants ----
            ident = cpool.tile([128, 128], f32, tag="ident")
            make_identity(nc, ident[:])
            ones_t = cpool.tile([128, 128], f32, tag="ones_t")
            nc.vector.memset(ones_t[:], 1.0)

            def col_rep(col8, tagn):
                """[8,1] column -> [128,8] partition-replicated row values."""
                dg = cpool.tile([8, 8], f32, tag=tagn + "_dg")
                nc.vector.tensor_tensor(out=dg[:], in0=ident[0:8, 0:8],
                                        in1=col8.to_broadcast([8, 8]), op=ALU.mult)
                pr = pstr.tile([128, 8], f32, tag="tr")
                nc.tensor.matmul(pr[:], ones_t[0:8, :], dg[:], start=True, stop=True)
                rep = cpool.tile([128, 8], f32, tag=tagn)
                nc.vector.tensor_copy(rep[:], pr[:])
                return rep

            def _b3(rep, nrep):
                return rep[:].unsqueeze(1).broadcast_to([128, nrep, 8])

            def row_T(row_ap, n128, tagout, dt=bf16):
                """[1, n128*128] fp32 row -> [128, n128] tile (dtype dt)."""
                o = cpool.tile([128, n128], dt, tag=tagout)
                for k in range(n128):
                    pt = pstr.tile([128, 1], f32, tag="tr")
                    nc.tensor.transpose(pt[:], row_ap[0:1, ts(k, 128)],
                                        ident[0:1, 0:1])
                    nc.vector.tensor_copy(o[:, k:k + 1], pt[:])
                return o

            # ---- big streaming loads (issued early) ----
            obs_sb = load((128, 32), obs, "obs", bf16)
            w1s = cpool.tile([128, 32 * 256], bf16, tag="w1s")
            dma(w1s[:].rearrange("p (k m) -> p k m", m=256),
                w1T.ap().rearrange("(k p) m -> p k m", p=128))
            ce_w2s = cpool.tile([128, 2 * HID], bf16, tag="ce_w2s")
            dma(ce_w2s[:].rearrange("p (k m) -> p k m", m=HID),
                ce_w2T.ap().rearrange("(k p) m -> p k m", p=128))
            qe_w1s = cpool.tile([128, 16 * 256], bf16, tag="qe_w1s")
            dma(qe_w1s[:].rearrange("p (k m) -> p k m", m=256),
                qe_w1T.ap().rearrange("(k p) m -> p k m", p=128))
            wq2s = cpool.tile([128, 2 * H2], bf16, tag="wq2s")
            dma(wq2s[:].rearrange("p (k m) -> p k m", m=H2),
                wq2T.ap().rearrange("(k p) m -> p k m", p=128))

            ce_b1_sb = load((1, 256), ce_b1r, "ce_b1")
            bn1sc_sb = load((1, 256), bn1_sc, "bn1sc")
            bn1sh_sb = load((1, 256), bn1_sh, "bn1sh")
            ce_b2_sb = load((128, 16), ce_b2, "ce_b2")
            ce_b2r_sb = load((1, H2), ce_b2r, "ce_b2r")
            qe_b1_sb = load((1, 256), qe_b1r, "qe_b1")
            qbias_sb = load((128, 8), qbias, "qbias")
            bk8_sb = load((128, 8), bk8, "bk8")
            bob_sb = load((128, 16), bob, "bob")
            mp_b1_sb = load((1, 256), mp_b1r, "mp_b1")
            bn2sc_sb = load((1, 256), bn2_sc, "bn2sc")
            bn2sh_sb = load((1, 256), bn2_sh, "bn2sh")
            mp_b2_sb = load((128, 8), mp_b2_8, "mp_b2")
            mp_b3_sb = load((4, 1), mp_b3, "mp_b3")
            oh8_sb = load((8, 1), oh8, "oh8")
            slot_sb = load((1, 5 * NCN), slot_mask, "slot")

            khs = cpool.tile([128, 8 * MPAD], f8, tag="khs")
            dma(khs[:].rearrange("p (j m) -> p j m", m=MPAD),
                khT.ap().rearrange("(j p) m -> p j m", p=128))
            woS = cpool.tile([128, 2 * HID], bf16, tag="woS")
            dma(woS[:].rearrange("p (k m) -> p k m", m=HID),
                woT.ap().rearrange("(k p) m -> p k m", p=128))
            mp1e_s = cpool.tile([128, 16 * 256], bf16, tag="mp1e_s")
            dma(mp1e_s[:].rearrange("p (k m) -> p k m", m=256),
                mp1eT.ap().rearrange("(k p) m -> p k m", p=128))
            mp1a_s = cpool.tile([128, 16 * 256], bf16, tag="mp1a_s")
            dma(mp1a_s[:].rearrange("p (k m) -> p k m", m=256),
                mp1aT.ap().rearrange("(k p) m -> p k m", p=128))
            mp1p_s = cpool.tile([128, 8 * 256], bf16, tag="mp1p_s")
            dma(mp1p_s[:].rearrange("p (k m) -> p k m", m=256),
                mp1pT.ap().rearrange("(k p) m -> p k m", p=128))
            mp_w2s = cpool.tile([128, 2 * H2], bf16, tag="mp_w2s")
            dma(mp_w2s[:].rearrange("p (k m) -> p k m", m=H2),
                mp_w2T.ap().rearrange("(k p) m -> p k m", p=128))
            mp_w3s = cpool.tile([128, 8 * 4], bf16, tag="mp_w3s")
            dma(mp_w3s[:].rearrange("p (k m) -> p k m", m=4),
                mp_w3T.ap().rearrange("(k p) m -> p k m", p=128))

            # ================= FRONT =================
            # L1: h_row = bn1(relu(ce_w1[b2] @ obs + b1))   [1, 256]
            ps_h = psx.tile([1, 256], f32, tag="px")
            for k in range(32):
                nc.tensor.matmul(ps_h[:], obs_sb[:, k:k + 1],
                                 w1s[:, ts(k, 256)],
                                 start=(k == 0), stop=(k == 31))
            h_row = cpool.tile([1, 256], f32, tag="h_row")
            nc.vector.tensor_add(h_row[:], ps_h[:], ce_b1_sb[:])
            nc.vector.tensor_scalar_max(h_row[:], h_row[:], 0.0)
            nc.vector.tensor_mul(h_row[:], h_row[:], bn1sc_sb[:])
            nc.vector.tensor_add(h_row[:], h_row[:], bn1sh_sb[:])
            if debug_taps:
                dma(dbg["dbg_h"].ap(), h_row[:])
            h_sb = row_T(h_row, 2, "h_sb")

            # L2: enc partial [1, 2048] = ce_w2[:, b2] @ h_c
            enc_p = cpool.tile([1, HID], f32, tag="rowst", name="enc_p")
            for nb in range(4):
                ps_e = pss.tile([1, 512], f32, tag="s")
                for kc in range(2):
                    nc.tensor.matmul(ps_e[:], h_sb[:, kc:kc + 1],
                                     ce_w2s[:, kc * HID + nb * 512:
                                            kc * HID + nb * 512 + 512],
                                     start=(kc == 0), stop=(kc == 1))
                nc.vector.tensor_copy(enc_p[:, ts(nb, 512)], ps_e[:])
            d_enc_i = dpool.tile([1, HID], f32, tag="d_enc_i")
            d_enc_o = dpool.tile([1, HID], f32, tag="d_enc_o")
            dma(d_enc_i[:], enc_p[:])
            nc.gpsimd.collective_compute("AllReduce", ALU.add, replica_groups=RG,
                                         ins=[d_enc_i.opt()], outs=[d_enc_o.opt()])
            enc_sb = cpool.tile([128, 16], f32, tag="enc_sb")
            dma(enc_sb[:], d_enc_o[:].rearrange("o (k p) -> (o p) k", p=128))
            enc_b = cpool.tile([128, 16], bf16, tag="enc_b")
            nc.vector.tensor_tensor(out=enc_b[:], in0=enc_sb[:],
                                    in1=ce_b2_sb[:], op=ALU.add)

            # query path: t = relu(qe_w1[b2] @ enc + b)    [1, 256]
            ps_t = psx.tile([1, 256], f32, tag="px")
            for k in range(16):
                nc.tensor.matmul(ps_t[:], enc_b[:, k:k + 1],
                                 qe_w1s[:, ts(k, 256)],
                                 start=(k == 0), stop=(k == 15))
            t_row = cpool.tile([1, 256], f32, tag="t_row")
            nc.vector.tensor_add(t_row[:], ps_t[:], qe_b1_sb[:])
            nc.vector.tensor_scalar_max(t_row[:], t_row[:], 0.0)
            if debug_taps:
                dma(dbg["dbg_t"].ap(), t_row[:])
            t_sb = row_T(t_row, 2, "t_sb")

            # qh partial [128, 8] = WQ2[:, tb2] @ t_c
            ps_qh = psx.tile([128, 8], f32, tag="px")
            for jm in range(8):
                for kc in range(2):
                    nc.tensor.matmul(ps_qh[:, jm:jm + 1],
                                     wq2s[:, kc * H2 + jm * 128:
                                          kc * H2 + jm * 128 + 128],
                                     t_sb[:, kc:kc + 1],
                                     start=(kc == 0), stop=(kc == 1))
            qh_p = cpool.tile([128, 8], f32, tag="qh_p")
            nc.vector.tensor_copy(qh_p[:], ps_qh[:])
            d_qh_i = dpool.tile([128, 8], f32, tag="d_qh_i")
            d_qh_o = dpool.tile([128, 8], f32, tag="d_qh_o")
            dma(d_qh_i[:], qh_p[:])
            nc.gpsimd.collective_compute("AllReduce", ALU.add, replica_groups=RG,
                                         ins=[d_qh_i.opt()], outs=[d_qh_o.opt()])
            nc.vector.tensor_add(enc_sb[:], enc_sb[:], ce_b2_sb[:])
            dma(out.ap()[10:2058].rearrange("(b p) -> p b", p=128), enc_sb[:])
            qh_sb = cpool.tile([128, 8], f32, tag="qh_sb")
            dma(qh_sb[:], d_qh_o[:])
            nc.vector.tensor_add(qh_sb[:], qh_sb[:], qbias_sb[:])
            nc.vector.tensor_scalar_mul(qh_sb[:], qh_sb[:], ISCALE)
            if debug_taps:
                dma(dbg["dbg_qh"].ap(), qh_sb[:])

            # masked per-stripe-pair stationaries (fp8, pre-scaled by S8).
            # DoubleRow layout: pair pj covers stripes j=2pj (slot i=0) and
            # j=2pj+1 (slot i=1); each slot is 16 cols (8 used + 8 pad).
            qkm = cpool.tile([128, 4 * 32], f8, tag="qkm")
            nc.vector.memset(qkm[:], 0.0)
            for j in range(8):
                pj, i = j // 2, j % 2
                dst = pj * 32 + i * 16 + j
                nc.vector.tensor_scalar_mul(qkm[:, dst:dst + 1],
                                            qh_sb[:, j:j + 1], S8)
            # c_h = bk . qh  (per-head scalar, already has ISCALE via qh)
            qb = cpool.tile([128, 8], f32, tag="qb")
            nc.vector.tensor_mul(qb[:], qh_sb[:], bk8_sb[:])
            ps_c = psx.tile([8, 1], f32, tag="px")
            nc.tensor.matmul(ps_c[:], qb[:], ones_t[:, 0:1], start=True, stop=True)
            c_sb = cpool.tile([8, 1], f32, tag="c_sb")
            nc.vector.tensor_copy(c_sb[:], ps_c[:])

            # ---- m1 stages A (enc) + P (proto) into one psum, staged to SBUF
            # (issued here; PE executes them while waiting on AR latencies)
            # proto block first (needs only enc)
            eb = cpool.tile([1, H2], f32, tag="eb")
            dma(eb[:], d_enc_o[0:1, 0:H2])
            nc.vector.tensor_add(eb[:], eb[:], ce_b2r_sb[:])
            pr_sb = cpool.tile([NPROTO, H2], f32, tag="protos")
            dma(pr_sb[:], protos.ap())
            dif = cpool.tile([NPROTO, H2], f32, tag="dif")
            for nb in range(2):
                ps_eb = pss.tile([NPROTO, 512], f32, tag="s")
                nc.tensor.matmul(ps_eb[:], ones_t[0:1, 0:NPROTO],
                                 eb[:, ts(nb, 512)], start=True, stop=True)
                nc.vector.tensor_tensor(out=dif[:, ts(nb, 512)],
                                        in0=pr_sb[:, ts(nb, 512)],
                                        in1=ps_eb[:], op=ALU.subtract)
            nc.vector.tensor_mul(dif[:], dif[:], dif[:])
            d2 = cpool.tile([NPROTO, 1], f32, tag="d2")
            nc.vector.tensor_reduce(out=d2[:], in_=dif[:], axis=AX.X, op=ALU.add)
            ptd = pstr.tile([1, 64], f32, tag="tr")
            nc.tensor.transpose(ptd[:], d2[:], ident[0:64, 0:64])
            dt_ = cpool.tile([1, 64], f32, tag="dt_")
            nc.vector.tensor_copy(dt_[:], ptd[:])
            dmin2 = cpool.tile([1, 1], f32, tag="dmin2")
            nc.vector.tensor_reduce(out=dmin2[:], in_=dt_[:], axis=AX.X, op=ALU.min)
            ps_dm = pstr.tile([NPROTO, 1], f32, tag="tr")
            nc.tensor.matmul(ps_dm[:], ones_t[0:1, 0:NPROTO], dmin2[:],
                             start=True, stop=True)
            oh64 = cpool.tile([NPROTO, 1], f32, tag="oh64")
            nc.vector.tensor_tensor(out=oh64[:], in0=d2[:],
                                    in1=ps_dm[:], op=ALU.is_equal)
            psel = cpool.tile([1, H2], f32, tag="psel")
            for nb in range(2):
                ps_ps = pss.tile([1, 512], f32, tag="s")
                nc.tensor.matmul(ps_ps[:], oh64[:],
                                 pr_sb[:, ts(nb, 512)], start=True, stop=True)
                nc.vector.tensor_copy(psel[:, ts(nb, 512)], ps_ps[:])
            dmin = cpool.tile([1, 1], f32, tag="dmin")
            nc.scalar.sqrt(dmin[:], dmin2[:])
            conf = cpool.tile([1, 1], f32, tag="conf")
            nc.vector.tensor_scalar_add(conf[:], dmin[:], 1.0)
            nc.vector.reciprocal(conf[:], conf[:])
            d_prow = dpool.tile([H2], f32, tag="d_prow")
            dma(d_prow[:].rearrange("(o b) -> o b", o=1), psel[:])
            ppad = cpool.tile([128, 16], f32, tag="ppad")
            nc.vector.memset(ppad[:], 0.0)
            dma(ppad[:, 0:8], d_prow[:].rearrange("(b p) -> p b", p=128))
            ppad_b = cpool.tile([128, 8], bf16, tag="ppad_b")
            nc.vector.tensor_copy(ppad_b[:], ppad[:, 0:8])
            dma(out.ap()[4106:6154].rearrange("(b p) -> p b", p=128), ppad[:])

            # m1 A + P partial
            ps_m1 = psx.tile([1, 256], f32, tag="px")
            for k in range(16):
                nc.tensor.matmul(ps_m1[:], enc_b[:, k:k + 1],
                                 mp1e_s[:, ts(k, 256)],
                                 start=(k == 0), stop=False)
            for k in range(8):
                nc.tensor.matmul(ps_m1[:], ppad_b[:, k:k + 1],
                                 mp1p_s[:, ts(k, 256)],
                                 start=False, stop=(k == 7))
            m1acc = cpool.tile([1, 256], f32, tag="m1acc")
            nc.vector.tensor_tensor(out=m1acc[:], in0=ps_m1[:],
                                    in1=mp_b1_sb[:], op=ALU.add)

            # ================= K-PASS =================
            # paired transposed weights: slot t2 covers m-tiles 2*t2, 2*t2+1
            wpair = cpool.tile([128, 25 * 32], f8, tag="wpair")
            nc.vector.memset(wpair[:], 0.0)
            zacc = cpool.tile([8, 16], f32, tag="zacc")
            nc.vector.memset(zacc[:], 0.0)
            for mc in range(NCH):
                cw = 512 if mc < 12 else 128
                ps_s = pss.tile([8, 512], f32, tag="s")
                khv = khs[:].rearrange("p (j m) -> p j m", m=MPAD)
                for pj in range(4):
                    nc.tensor.matmul(
                        ps_s[:, 0:cw],
                        qkm[:, pj * 32:(pj + 1) * 32]
                        .rearrange("p (i h) -> p i h", i=2)[:, :, 0:8],
                        khv[:, 2 * pj:2 * pj + 2, mc * 512:mc * 512 + cw],
                        start=(pj == 0), stop=(pj == 3),
                        perf_mode=mybir.MatmulPerfMode.DoubleRow)
                w_c = cpool.tile([8, 512], f32, tag="w_c")
                if mc < 12:
                    nc.scalar.activation(w_c[:, 0:cw], ps_s[:, 0:cw], ACTF.Exp,
                                         bias=c_sb[:], scale=1.0 / S8,
                                         accum_out=zacc[:, mc:mc + 1])
                else:
                    nc.scalar.activation(w_c[:, 0:cw], ps_s[:, 0:cw], ACTF.Exp,
                                         bias=c_sb[:], scale=1.0 / S8)
                    nc.vector.memset(w_c[:, MVALID_TAIL:cw], 0.0)
                    nc.vector.tensor_reduce(out=zacc[:, mc:mc + 1],
                                            in_=w_c[:, 0:cw], axis=AX.X,
                                            op=ALU.add)
                if debug_taps and mc == 0:
                    dma(dbg["dbg_w0"].ap(), w_c[:])
                for ti in range(cw // 128):
                    gt = mc * 4 + ti
                    pt = pstr.tile([128, 8], f32, tag="tr")
                    nc.tensor.transpose(pt[:], w_c[0:8, ts(ti, 128)],
                                        ident[0:8, 0:8])
                    dst = (gt // 2) * 32 + (gt % 2) * 16
                    nc.vector.tensor_copy(wpair[:, dst:dst + 8], pt[:])

            # local Z per head
            zloc = cpool.tile([8, 1], f32, tag="zloc")
            nc.vector.tensor_reduce(out=zloc[:], in_=zacc[:, 0:NCH], axis=AX.X,
                                    op=ALU.add)
            if debug_taps:
                dma(dbg["dbg_z"].ap(), zacc[:])

            # ---- top-5 candidates (overlaps V-pass) ----
            rz8 = cpool.tile([8, 1], f32, tag="rz8")
            nc.vector.reciprocal(rz8[:], zloc[:])
            nc.vector.tensor_scalar_mul(rz8[:], rz8[:], 1.0 / (NH * NCN))
            zq = col_rep(rz8[:], "zq")
            awf = cpool.tile([128, 50 * 8], f32, tag="awf")
            nc.vector.tensor_tensor(
                out=awf[:].rearrange("p (a b) -> p a b", b=8),
                in0=wpair[:].rearrange("p (a g b) -> p a g b", g=2, b=8)[:, :, 0, :],
                in1=_b3(zq, 50), op=ALU.mult)
            attnw = cpool.tile([128, 50], f32, tag="attnw")
            nc.vector.tensor_reduce(out=attnw[:],
                                    in_=awf[:].rearrange("p (a b) -> p a b", b=8),
                                    axis=AX.X, op=ALU.add)
            cand1 = cpool.tile([128, 8], f32, tag="cand1")
            nc.vector.max(out=cand1[:], in_=attnw[:])
            ptc1 = pstr.tile([8, 128], f32, tag="tr")
            nc.tensor.transpose(ptc1[:], cand1[:], ident[:, :])
            cd2 = cpool.tile([8, 128], f32, tag="cd2")
            nc.vector.tensor_copy(cd2[:], ptc1[:])
            cand2 = cpool.tile([8, 8], f32, tag="cand2")
            nc.vector.max(out=cand2[:], in_=cd2[:])
            d_c64 = dpool.tile([64], f32, tag="d_c64")
            dma(d_c64[:].rearrange("(p b) -> p b", b=8), cand2[:])
            c64 = cpool.tile([1, 64], f32, tag="c64")
            dma(c64[:], d_c64[:].rearrange("(o b) -> o b", o=1))
            top8 = cpool.tile([1, 8], f32, tag="top8")
            nc.vector.max(out=top8[:], in_=c64[:])
            slots = cpool.tile([1, 5 * NCN], f32, tag="slots")
            for i in range(NCN):
                nc.vector.tensor_copy(slots[:, i * 5:(i + 1) * 5], top8[:, 0:5])
            nc.vector.tensor_mul(slots[:], slots[:], slot_sb[:])

            # ================= V-PASS =================
            ps_u = []
            for nb in range(4):
                ps_unb = psu.tile([8, 512], f32, tag=f"u{nb}", name=f"ps_u{nb}")
                ps_u.append(ps_unb)
            for cd in range(NCH):
                ntile = 4 if cd < 12 else 1
                rows = 512 if cd < 12 else 128
                vt = vpool.tile([128, 4 * HID], f8, tag="vt")
                dma(vt[:, 0:ntile * HID].rearrange("p (mc d) -> p mc d", d=HID),
                    vh.ap()[cd * 512: cd * 512 + rows, :]
                    .rearrange("(mc p) d -> p mc d", p=128))
                vtv = vt[:].rearrange("p (mc d) -> p mc d", d=HID)
                if cd < 12:
                    for t2l in range(2):
                        t2 = cd * 2 + t2l
                        for nb in range(4):
                            nc.tensor.matmul(
                                ps_u[nb][:],
                                wpair[:, t2 * 32:(t2 + 1) * 32]
                                .rearrange("p (i h) -> p i h", i=2)[:, :, 0:8],
                                vtv[:, 2 * t2l:2 * t2l + 2,
                                    nb * 512:nb * 512 + 512],
                                start=(t2 == 0), stop=False,
                                perf_mode=mybir.MatmulPerfMode.DoubleRow)
                else:
                    for nb in range(4):
                        nc.tensor.matmul(ps_u[nb][:],
                                         wpair[:, 24 * 32:24 * 32 + 8],
                                         vt[:, nb * 512:nb * 512 + 512],
                                         start=False, stop=True)

            # ---- u AllReduce payload: [8, 2048 u | 1 Z | 40 slots | 7 pad]
            UW = 3896
            d_u_i = dpool.tile([8, UW], bf16, tag="d_u_i")
            d_u_o = dpool.tile([8, UW], bf16, tag="d_u_o")
            u_s = cpool.tile([8, HID], bf16, tag="u_s")
            for nb in range(4):
                nc.vector.tensor_copy(u_s[:, ts(nb, 512)], ps_u[nb][:])
            dma(d_u_i[:].rearrange("h w -> (h w)")[ds(1792, 8 * (UW - 256))]
                .rearrange("(h d) -> h d", d=UW - 256)[:, 0:HID],
                u_s[:])
            stg = cpool.tile([8, 48], bf16, tag="stg")
            nc.vector.memset(stg[:], 0.0)
            nc.vector.tensor_copy(stg[:, 0:1], zloc[:])
            nc.vector.tensor_copy(stg[0:1, 1:41], slots[:])
            dma(d_u_i[:, 3840:3888], stg[:])
            nc.gpsimd.collective_compute("AllReduce", ALU.add, replica_groups=RG,
                                         ins=[d_u_i.opt()], outs=[d_u_o.opt()])

            # ---- post-AR: Z, top5, ctx extraction ----
            G = cpool.tile([8, 48], bf16, tag="G")
            dma(G[:], d_u_o[:, 3840:3888])
            zg = cpool.tile([8, 1], f32, tag="zg")
            nc.vector.reciprocal(zg[:], G[:, 0:1])
            top40 = cpool.tile([1, 5 * NCN], f32, tag="top40")
            nc.vector.tensor_copy(top40[:], G[0:1, 1:41])
            top8f = cpool.tile([1, 8], f32, tag="top8f")
            nc.vector.max(out=top8f[:], in_=top40[:])

            ctxm_b = cpool.tile([8, 256], bf16, tag="ctxm_b")
            dma(ctxm_b[:], d_u_o[:, 1792:2048])
            ctxm = cpool.tile([8, 256], f32, tag="ctxm")
            nc.vector.tensor_scalar(out=ctxm[:], in0=ctxm_b[:], scalar1=zg[:],
                                    scalar2=None, op0=ALU.mult)
            if debug_taps:
                dma(dbg["dbg_u"].ap(),
                    d_u_o[:].rearrange("h w -> (h w)")[ds(1792, 8 * (UW - 256))]
                    .rearrange("(h d) -> h d", d=UW - 256)[:, 0:HID])
            ps_cr = psx.tile([1, 256], f32, tag="px")
            nc.tensor.matmul(ps_cr[:], oh8_sb[:], ctxm[:], start=True, stop=True)
            ctx_row = cpool.tile([1, 256], f32, tag="ctx_row")
            nc.vector.tensor_copy(ctx_row[:], ps_cr[:])
            ctx_sb = row_T(ctx_row, 2, "ctx_sb")
            # full ctx in p-major layout for the folded m1-attended stage:
            # col k holds ctx[k*128 : (k+1)*128] = ctxm[k//2, (k%2)*128 : +128]
            ctx_pm = cpool.tile([128, 16], bf16, tag="ctx_pm")
            ctxT = cpool.tile([128, 16], f32, tag="ctxT")
            for j in range(2):
                ptc = pstr.tile([128, 8], f32, tag="tr")
                nc.tensor.transpose(ptc[:], ctxm[0:8, j * 128:(j + 1) * 128],
                                    ident[0:8, 0:8])
                nc.vector.tensor_copy(ctxT[:, ts(j, 8)], ptc[:])
            for k in range(16):
                nc.vector.tensor_copy(ctx_pm[:, k:k + 1],
                                      ctxT[:, (k % 2) * 8 + k // 2:
                                           (k % 2) * 8 + k // 2 + 1])
            if debug_taps:
                dma(dbg["dbg_ctx"].ap(), ctx_sb[:])

            # attended partial = wo[:, b2] @ ctx_c   (row orientation)
            att_p = cpool.tile([1, HID], f32, tag="rowst", name="att_p")
            for nb in range(4):
                ps_at = pss.tile([1, 512], f32, tag="s")
                for kc in range(2):
                    nc.tensor.matmul(ps_at[:], ctx_sb[:, kc:kc + 1],
                                     woS[:, kc * HID + nb * 512:
                                         kc * HID + nb * 512 + 512],
                                     start=(kc == 0), stop=(kc == 1))
                nc.vector.tensor_copy(att_p[:, ts(nb, 512)], ps_at[:])
            d_a_i = dpool.tile([1, HID], f32, tag="d_a_i")
            d_a_o = dpool.tile([1, HID], f32, tag="d_a_o")
            dma(d_a_i[:], att_p[:])
            # m1 stage B from folded (mp1a @ wo) @ ctx — independent of att-AR
            ps_m1b = psx.tile([1, 256], f32, tag="px")
            for k in range(16):
                nc.tensor.matmul(ps_m1b[:], ctx_pm[:, k:k + 1],
                                 mp1a_s[:, ts(k, 256)],
                                 start=(k == 0), stop=(k == 15))
            m1_row = cpool.tile([1, 256], f32, tag="m1_row")
            nc.vector.tensor_add(m1_row[:], ps_m1b[:], m1acc[:])
            nc.vector.tensor_scalar_max(m1_row[:], m1_row[:], 0.0)
            nc.vector.tensor_mul(m1_row[:], m1_row[:], bn2sc_sb[:])
            nc.vector.tensor_add(m1_row[:], m1_row[:], bn2sh_sb[:])
            if debug_taps:
                dma(dbg["dbg_m1"].ap(), m1_row[:])
            m1_sb = row_T(m1_row, 2, "m1_sb")

            # m2 partial = mp_w2[:, b2] @ m1_c   (row orientation)
            m2_pf = cpool.tile([1, HID], f32, tag="rowst", name="m2_pf")
            m2_p = m2_pf[0:1, 0:H2]
            for nb in range(2):
                ps_m2 = pss.tile([1, 512], f32, tag="s")
                for kc in range(2):
                    nc.tensor.matmul(ps_m2[:], m1_sb[:, kc:kc + 1],
                                     mp_w2s[:, kc * H2 + nb * 512:
                                            kc * H2 + nb * 512 + 512],
                                     start=(kc == 0), stop=(kc == 1))
                nc.vector.tensor_copy(m2_p[:, ts(nb, 512)], ps_m2[:])
            d_m2_i = dpool.tile([1, H2], f32, tag="d_m2_i")
            d_m2_o = dpool.tile([1, H2], f32, tag="d_m2_o")
            dma(d_m2_i[:], m2_p)
            nc.gpsimd.collective_compute("AllReduce", ALU.add, replica_groups=RG,
                                         ins=[d_m2_i.opt()], outs=[d_m2_o.opt()])
            nc.gpsimd.collective_compute("AllReduce", ALU.add, replica_groups=RG,
                                         ins=[d_a_i.opt()], outs=[d_a_o.opt()])
            m2_sb = cpool.tile([128, 8], f32, tag="m2_sb")
            dma(m2_sb[:], d_m2_o[:].rearrange("o (k p) -> (o p) k", p=128))
            nc.vector.tensor_add(m2_sb[:], m2_sb[:], mp_b2_sb[:])
            nc.vector.tensor_scalar_max(m2_sb[:], m2_sb[:], 0.0)
            m2_b = cpool.tile([128, 8], bf16, tag="m2_b")
            nc.vector.tensor_copy(m2_b[:], m2_sb[:])

            # meta = mp_w3 @ m2 + b3
            ps_mt = psx.tile([4, 1], f32, tag="px")
            for k in range(8):
                nc.tensor.matmul(ps_mt[:], mp_w3s[:, ts(k, 4)],
                                 m2_b[:, k:k + 1],
                                 start=(k == 0), stop=(k == 7))
            meta_sb = cpool.tile([4, 1], f32, tag="meta_sb")
            nc.vector.tensor_add(meta_sb[:], ps_mt[:], mp_b3_sb[:])
            ptmt = pstr.tile([1, 4], f32, tag="tr")
            nc.tensor.transpose(ptmt[:], meta_sb[:], ident[0:4, 0:4])
            metaT = cpool.tile([1, 4], f32, tag="metaT")
            nc.vector.tensor_copy(metaT[:], ptmt[:])
            nmax = cpool.tile([1, 1], f32, tag="nmax")
            nc.vector.tensor_reduce(out=nmax[:], in_=metaT[:, 0:3], axis=AX.X,
                                    op=ALU.max)
            nc.vector.tensor_scalar_mul(nmax[:], nmax[:], -1.0)
            e3 = cpool.tile([1, 3], f32, tag="e3")
            nc.scalar.activation(e3[:], metaT[:, 0:3], ACTF.Exp, bias=nmax[:])
            s3 = cpool.tile([1, 1], f32, tag="s3")
            nc.vector.tensor_reduce(out=s3[:], in_=e3[:], axis=AX.X, op=ALU.add)
            nc.vector.reciprocal(s3[:], s3[:])
            regime = cpool.tile([1, 3], f32, tag="regime")
            nc.vector.tensor_scalar(out=regime[:], in0=e3[:], scalar1=s3[:],
                                    scalar2=None, op0=ALU.mult)
            crisis = cpool.tile([1, 1], f32, tag="crisis")
            nc.scalar.activation(crisis[:], metaT[:, 3:4], ACTF.Sigmoid)

            att_f = cpool.tile([128, 16], f32, tag="att_f")
            dma(att_f[:], d_a_o[:].rearrange("o (k p) -> (o p) k", p=128))
            nc.vector.tensor_add(att_f[:], att_f[:], bob_sb[:])
            dma(out.ap()[2058:4106].rearrange("(b p) -> p b", p=128), att_f[:])

            # ---- output assembly ----
            dma(out.ap()[0:3].rearrange("(o b) -> o b", o=1), regime[:])
            dma(out.ap()[3:4].rearrange("(o b) -> o b", o=1), crisis[:])
            dma(out.ap()[4:5].rearrange("(o b) -> o b", o=1), conf[:])
            dma(out.ap()[5:10].rearrange("(o b) -> o b", o=1), top8f[:, 0:5])

    nc.compile()
    return nc


_NC_CACHE = {}


def _get_nc():
    if "nc" not in _NC_CACHE:
        _NC_CACHE["nc"] = build_nc()
    return _NC_CACHE["nc"]


def _bm(x, nb):
    """vector [nb*128] -> b-major [128, nb] (col b = x[b*128:(b+1)*128])."""
    return np.ascontiguousarray(np.asarray(x, np.float32).reshape(nb, 128).T)


def _bf(x):
    return np.ascontiguousarray(np.asarray(x)).astype(ml_dtypes.bfloat16)


def _f8(x):
    return np.ascontiguousarray(np.asarray(x)).astype(ml_dtypes.float8_e4m3)


def shard_inputs(i):
    g = {k: np.asarray(v, np.float32) for k, v in i.items()}
    # host folds
    kh = g["memory_keys"] @ g["wk"].T                       # [MEM, H2]
    vhf = g["memory_values"] @ g["wv"].T                    # [MEM, HID]
    WQ2 = g["wq"] @ g["qe_w2"]                              # [H2, HID]
    qbias_full = g["wq"] @ g["qe_b2"] + g["bq"]             # [H2]
    bob_full = g["wo"] @ g["bv"] + g["bo"]                  # [HID]
    bn1_scale = g["bn1_g"] / np.sqrt(g["bn1_v"] + EPS)
    bn1_shift = g["bn1_b"] - g["bn1_m"] * bn1_scale
    bn2_scale = g["bn2_g"] / np.sqrt(g["bn2_v"] + EPS)
    bn2_shift = g["bn2_b"] - g["bn2_m"] * bn2_scale

    in_maps = []
    for c in range(NCN):
        b2 = slice(c * 256, (c + 1) * 256)
        khp = np.zeros((MPAD, H2), np.float32)
        khp[0:MSH] = kh[c * MSH:(c + 1) * MSH]
        vhp = np.zeros((MPAD, HID), np.float32)
        vhp[0:MSH] = vhf[c * MSH:(c + 1) * MSH]
        oh = np.zeros((8, 1), np.float32); oh[c, 0] = 1.0
        sm = np.zeros((1, 40), np.float32); sm[0, c * 5:(c + 1) * 5] = 1.0
        m = {
            "obs": _bf(_bm(g["observation"], 32)),
            "w1T": _bf(g["ce_w1"][b2].T),
            "ce_b1r": g["ce_b1"][b2].reshape(1, 256),
            "bn1_sc": bn1_scale[b2].reshape(1, 256),
            "bn1_sh": bn1_shift[b2].reshape(1, 256),
            "ce_w2T": _bf(g["ce_w2"][:, b2].T),
            "ce_b2": _bm(g["ce_b2"], 16),
            "ce_b2r": g["ce_b2"][0:H2].reshape(1, H2),
            "qe_w1T": _bf(g["qe_w1"][b2].T),
            "qe_b1r": g["qe_b1"][b2].reshape(1, 256),
            "wq2T": _bf(WQ2[:, b2].T),
            "qbias": _bm(qbias_full, 8),
            "bk8": _bm(g["bk"], 8),
            "khT": _f8(khp.T),
            "vh": _f8(vhp),
            "woT": _bf(g["wo"][:, b2].T),
            "bob": _bm(bob_full, 16),
            "protos": np.ascontiguousarray(g["prototypes"]),
            "mp1eT": _bf(g["mp_w1"][b2, 0:HID].T),
            "mp1aT": _bf((g["mp_w1"][b2, HID:2 * HID] @ g["wo"]).T),
            "mp1pT": _bf(g["mp_w1"][b2, 2 * HID:2 * HID + H2].T),
            "mp_b1r": (g["mp_b1"][b2]
                       + g["mp_w1"][b2, HID:2 * HID] @ bob_full
                       ).reshape(1, 256),
            "bn2_sc": bn2_scale[b2].reshape(1, 256),
            "bn2_sh": bn2_shift[b2].reshape(1, 256),
            "mp_w2T": _bf(g["mp_w2"][:, b2].T),
            "mp_b2_8": _bm(g["mp_b2"], 8),
            "mp_w3T": _bf(g["mp_w3"].T),
            "mp_b3": np.asarray(g["mp_b3"], np.float32).reshape(4, 1).copy(),
            "oh8": oh,
            "slot_mask": sm,
        }
        in_maps.append(m)
    return in_maps


def kernel(**inputs):
    nc = _get_nc()
    in_maps = shard_inputs(inputs)
    res = bass_utils.run_bass_kernel_spmd(nc, in_maps, core_ids=list(range(NCN)))
    return np.asarray(res.results[0]["out"], np.float32)


# revision 23
# speedup vs baseline: 1.0073x; 1.0073x over previous
"""Trainium2 Bass kernel for nn_MetaLearningCrisisMemory (retrieval_knn).

Self-contained: kernel(**inputs) -> np.ndarray [6154] fp32.

v2 strategy (8-way SPMD, memory-bound target):
 - Host-fold wk into K (kh = K @ wk.T) and wv into V (vh = V @ wv.T): the
   two big device passes become pure streamed sweeps. kh/vh shipped fp8
   (e4m3); output-norm analysis shows the attended section carries ~0.07%
   of output norm^2, so fp8 noise there is negligible.
 - Scores bounded (~|1.3|): exp without max-subtraction; softmax
   normalization Z rides along the u-AllReduce. No flash-max machinery.
 - Matmuls in vector-stationary orientation with N=512 moving columns:
   ~500 PE instructions total (vs 3225 in v1 at a fixed ~213ns each).
 - 5 AllReduces: enc, qh, u(+Z+top5 slots), attended, m2.
 - All small Linears tensor-parallel with bf16 host-pre-transposed shards.
"""

import numpy as np
import ml_dtypes

import concourse.bass as bass
import concourse.mybir as mybir
import concourse.tile as tile
from concourse import bacc, bass_utils
from concourse.bass import ts, ds
from concourse.masks import make_identity

f32 = mybir.dt.float32
bf16 = mybir.dt.bfloat16
f8 = mybir.dt.float8e4
AX = mybir.AxisListType
ALU = mybir.AluOpType
ACTF = mybir.ActivationFunctionType

NCN = 8
INPUT_DIM, HID, MEM, NPROTO = 4096, 2048, 50000, 64
H2 = HID // 2                  # 1024
NH = 8
DQ = H2 // NH                  # 128
DV = HID // NH                 # 256
TOPK = 5
EPS = 1e-5
MSH = MEM // NCN               # 6250 rows per core
MPAD = 6272                    # padded to 49 * 128
MT = MPAD // 128               # 49 m-tiles
NCH = 13                       # 12 chunks of 512 + 1 of 128
MVALID_TAIL = 106              # valid rows in tile 48 (6250 - 48*128)
OUT_N = 3 + 1 + 1 + TOPK + 3 * HID  # 6154
ISCALE = 1.0 / float(np.sqrt(np.float32(DQ)))
S8 = 32.0                      # fp8 pre-scale for the query


def _din(nc, name, shape, dt=f32):
    return nc.dram_tensor(name, list(shape), dt, kind="ExternalInput")


def build_nc(debug_taps=False):
    nc = bacc.Bacc("TRN2", target_bir_lowering=False, debug=False,
                   enable_asserts=False, num_devices=NCN)

    # ---- I/O ----
    obs = _din(nc, "obs", (128, 32), bf16)
    w1T = _din(nc, "w1T", (INPUT_DIM, 256), bf16)
    ce_b1r = _din(nc, "ce_b1r", (1, 256))
    bn1_sc = _din(nc, "bn1_sc", (1, 256))
    bn1_sh = _din(nc, "bn1_sh", (1, 256))
    ce_w2T = _din(nc, "ce_w2T", (256, HID), bf16)
    ce_b2 = _din(nc, "ce_b2", (128, 16))
    ce_b2r = _din(nc, "ce_b2r", (1, H2))
    qe_w1T = _din(nc, "qe_w1T", (HID, 256), bf16)
    qe_b1r = _din(nc, "qe_b1r", (1, 256))
    wq2T = _din(nc, "wq2T", (256, H2), bf16)
    qbias = _din(nc, "qbias", (128, 8))
    bk8 = _din(nc, "bk8", (128, 8))
    khT = _din(nc, "khT", (H2, MPAD), f8)
    vh = _din(nc, "vh", (MPAD, HID), f8)
    woT = _din(nc, "woT", (256, HID), bf16)
    bob = _din(nc, "bob", (128, 16))
    protos = _din(nc, "protos", (NPROTO, H2))
    mp1eT = _din(nc, "mp1eT", (HID, 256), bf16)
    mp1aT = _din(nc, "mp1aT", (HID, 256), bf16)
    mp1pT = _din(nc, "mp1pT", (H2, 256), bf16)
    mp_b1r = _din(nc, "mp_b1r", (1, 256))
    bn2_sc = _din(nc, "bn2_sc", (1, 256))
    bn2_sh = _din(nc, "bn2_sh", (1, 256))
    mp_w2T = _din(nc, "mp_w2T", (256, H2), bf16)
    mp_b2_8 = _din(nc, "mp_b2_8", (128, 8))
    mp_w3T = _din(nc, "mp_w3T", (H2, 4), bf16)
    mp_b3 = _din(nc, "mp_b3", (4, 1))
    oh8 = _din(nc, "oh8", (8, 1))
    slot_mask = _din(nc, "slot_mask", (1, 5 * NCN))
    out = nc.dram_tensor("out", [OUT_N], f32, kind="ExternalOutput")
    dbg = {}
    if debug_taps:
        for nm, shp in (("dbg_w0", [8, 512]), ("dbg_qh", [128, 8]),
                        ("dbg_u", [8, HID]), ("dbg_ctx", [128, 2]),
                        ("dbg_z", [8, 16]), ("dbg_m1", [1, 256]),
                        ("dbg_h", [1, 256]), ("dbg_t", [1, 256])):
            dbg[nm] = nc.dram_tensor(nm, shp, f32, kind="ExternalOutput")

    RG = [list(range(NCN))]

    with tile.TileContext(nc) as tc:
        import contextlib
        with contextlib.ExitStack() as stk:
            cpool = stk.enter_context(tc.tile_pool(name="cpool", bufs=1))
            vpool = stk.enter_context(tc.tile_pool(name="vpool", bufs=3))
            psx = stk.enter_context(tc.tile_pool(name="psx", bufs=1, space="PSUM"))
            pss = stk.enter_context(tc.tile_pool(name="pss", bufs=2, space="PSUM"))
            pstr = stk.enter_context(tc.tile_pool(name="pstr", bufs=1, space="PSUM"))
            psu = stk.enter_context(tc.tile_pool(name="psu", bufs=1, space="PSUM"))
            dpool = stk.enter_context(tc.tile_pool(name="dpool", bufs=1, space="DRAM"))

            def dma(dst, src):
                nc.sync.dma_start(out=dst, in_=src)

            def load(shape, dram_t, tag, dt=f32):
                t = cpool.tile(list(shape), dt, tag=tag)
                dma(t[:], dram_t.ap())
                return t

            # ---- CC warm-up first: pulls the CC entry barrier to t~0 and
            # absorbs the cold-stream cost under the front compute
            d_w_i = dpool.tile([1, 8], f32, tag="d_w_i")
            d_w_o = dpool.tile([1, 8], f32, tag="d_w_o")
            nc.gpsimd.collective_compute("AllReduce", ALU.add, replica_groups=RG,
                                         ins=[d_w_i.opt()], outs=[d_w_o.opt()])

            # ---- constants ----
            ident = cpool.tile([128, 128], f32, tag="ident")
            make_identity(nc, ident[:])
            ones_t = cpool.tile([128, 128], f32, tag="ones_t")
            nc.vector.memset(ones_t[:], 1.0)

            def col_rep(col8, tagn):
                """[8,1] column -> [128,8] partition-replicated row values."""
                dg = cpool.tile([8, 8], f32, tag=tagn + "_dg")
                nc.vector.tensor_tensor(out=dg[:], in0=ident[0:8, 0:8],
                                        in1=col8.to_broadcast([8, 8]), op=ALU.mult)
                pr = pstr.tile([128, 8], f32, tag="tr")
                nc.tensor.matmul(pr[:], ones_t[0:8, :], dg[:], start=True, stop=True)
                rep = cpool.tile([128, 8], f32, tag=tagn)
                nc.vector.tensor_copy(rep[:], pr[:])
                return rep

            def _b3(rep, nrep):
                return rep[:].unsqueeze(1).broadcast_to([128, nrep, 8])

            def row_T(row_ap, n128, tagout, dt=bf16):
                """[1, n128*128] fp32 row -> [128, n128] tile (dtype dt)."""
                o = cpool.tile([128, n128], dt, tag=tagout)
                for k in range(n128):
                    pt = pstr.tile([128, 1], f32, tag="tr")
                    nc.tensor.transpose(pt[:], row_ap[0:1, ts(k, 128)],
                                        ident[0:1, 0:1])
                    nc.vector.tensor_copy(o[:, k:k + 1], pt[:])
                return o

            # ---- big streaming loads (issued early) ----
            obs_sb = load((128, 32), obs, "obs", bf16)
            w1s = cpool.tile([128, 32 * 256], bf16, tag="w1s")
            dma(w1s[:].rearrange("p (k m) -> p k m", m=256),
                w1T.ap().rearrange("(k p) m -> p k m", p=128))
            ce_w2s = cpool.tile([128, 2 * HID], bf16, tag="ce_w2s")
            dma(ce_w2s[:].rearrange("p (k m) -> p k m", m=HID),
                ce_w2T.ap().rearrange("(k p) m -> p k m", p=128))
            qe_w1s = cpool.tile([128, 16 * 256], bf16, tag="qe_w1s")
            dma(qe_w1s[:].rearrange("p (k m) -> p k m", m=256),
                qe_w1T.ap().rearrange("(k p) m -> p k m", p=128))
            wq2s = cpool.tile([128, 2 * H2], bf16, tag="wq2s")
            dma(wq2s[:].rearrange("p (k m) -> p k m", m=H2),
                wq2T.ap().rearrange("(k p) m -> p k m", p=128))

            ce_b1_sb = load((1, 256), ce_b1r, "ce_b1")
            bn1sc_sb = load((1, 256), bn1_sc, "bn1sc")
            bn1sh_sb = load((1, 256), bn1_sh, "bn1sh")
            ce_b2_sb = load((128, 16), ce_b2, "ce_b2")
            ce_b2r_sb = load((1, H2), ce_b2r, "ce_b2r")
            qe_b1_sb = load((1, 256), qe_b1r, "qe_b1")
            qbias_sb = load((128, 8), qbias, "qbias")
            bk8_sb = load((128, 8), bk8, "bk8")
            bob_sb = load((128, 16), bob, "bob")
            mp_b1_sb = load((1, 256), mp_b1r, "mp_b1")
            bn2sc_sb = load((1, 256), bn2_sc, "bn2sc")
            bn2sh_sb = load((1, 256), bn2_sh, "bn2sh")
            mp_b2_sb = load((128, 8), mp_b2_8, "mp_b2")
            mp_b3_sb = load((4, 1), mp_b3, "mp_b3")
            oh8_sb = load((8, 1), oh8, "oh8")
            slot_sb = load((1, 5 * NCN), slot_mask, "slot")

            khs = cpool.tile([128, 8 * MPAD], f8, tag="khs")
            dma(khs[:].rearrange("p (j m) -> p j m", m=MPAD),
                khT.ap().rearrange("(j p) m -> p j m", p=128))
            woS = cpool.tile([128, 2 * HID], bf16, tag="woS")
            dma(woS[:].rearrange("p (k m) -> p k m", m=HID),
                woT.ap().rearrange("(k p) m -> p k m", p=128))
            mp1e_s = cpool.tile([128, 16 * 256], bf16, tag="mp1e_s")
            dma(mp1e_s[:].rearrange("p (k m) -> p k m", m=256),
                mp1eT.ap().rearrange("(k p) m -> p k m", p=128))
            mp1a_s = cpool.tile([128, 16 * 256], bf16, tag="mp1a_s")
            dma(mp1a_s[:].rearrange("p (k m) -> p k m", m=256),
                mp1aT.ap().rearrange("(k p) m -> p k m", p=128))
            mp1p_s = cpool.tile([128, 8 * 256], bf16, tag="mp1p_s")
            dma(mp1p_s[:].rearrange("p (k m) -> p k m", m=256),
                mp1pT.ap().rearrange("(k p) m -> p k m", p=128))
            mp_w2s = cpool.tile([128, 2 * H2], bf16, tag="mp_w2s")
            dma(mp_w2s[:].rearrange("p (k m) -> p k m", m=H2),
                mp_w2T.ap().rearrange("(k p) m -> p k m", p=128))
            mp_w3s = cpool.tile([128, 8 * 4], bf16, tag="mp_w3s")
            dma(mp_w3s[:].rearrange("p (k m) -> p k m", m=4),
                mp_w3T.ap().rearrange("(k p) m -> p k m", p=128))

            # ================= FRONT =================
            # L1: h_row = bn1(relu(ce_w1[b2] @ obs + b1))   [1, 256]
            ps_h = psx.tile([1, 256], f32, tag="px")
            for k in range(32):
                nc.tensor.matmul(ps_h[:], obs_sb[:, k:k + 1],
                                 w1s[:, ts(k, 256)],
                                 start=(k == 0), stop=(k == 31))
            h_row = cpool.tile([1, 256], f32, tag="h_row")
            nc.vector.tensor_add(h_row[:], ps_h[:], ce_b1_sb[:])
            nc.vector.tensor_scalar_max(h_row[:], h_row[:], 0.0)
            nc.vector.tensor_mul(h_row[:], h_row[:], bn1sc_sb[:])
            nc.vector.tensor_add(h_row[:], h_row[:], bn1sh_sb[:])
            if debug_taps:
                dma(dbg["dbg_h"].ap(), h_row[:])
            h_sb = row_T(h_row, 2, "h_sb")

            # L2: enc partial [1, 2048] = ce_w2[:, b2] @ h_c
            enc_p = cpool.tile([1, HID], f32, tag="rowst", name="enc_p")
            for nb in range(4):
                ps_e = pss.tile([1, 512], f32, tag="s")
                for kc in range(2):
                    nc.tensor.matmul(ps_e[:], h_sb[:, kc:kc + 1],
                                     ce_w2s[:, kc * HID + nb * 512:
                                            kc * HID + nb * 512 + 512],
                                     start=(kc == 0), stop=(kc == 1))
                nc.vector.tensor_copy(enc_p[:, ts(nb, 512)], ps_e[:])
            d_enc_i = dpool.tile([1, HID], f32, tag="d_enc_i")
            d_enc_o = dpool.tile([1, HID], f32, tag="d_enc_o")
            dma(d_enc_i[:], enc_p[:])
            nc.gpsimd.collective_compute("AllReduce", ALU.add, replica_groups=RG,
                                         ins=[d_enc_i.opt()], outs=[d_enc_o.opt()])
            enc_sb = cpool.tile([128, 16], f32, tag="enc_sb")
            dma(enc_sb[:], d_enc_o[:].rearrange("o (k p) -> (o p) k", p=128))
            enc_b = cpool.tile([128, 16], bf16, tag="enc_b")
            nc.vector.tensor_tensor(out=enc_b[:], in0=enc_sb[:],
                                    in1=ce_b2_sb[:], op=ALU.add)

            # query path: t = relu(qe_w1[b2] @ enc + b)    [1, 256]
            ps_t = psx.tile([1, 256], f32, tag="px")
            for k in range(16):
                nc.tensor.matmul(ps_t[:], enc_b[:, k:k + 1],
                                 qe_w1s[:, ts(k, 256)],
                                 start=(k == 0), stop=(k == 15))
            t_row = cpool.tile([1, 256], f32, tag="t_row")
            nc.vector.tensor_add(t_row[:], ps_t[:], qe_b1_sb[:])
            nc.vector.tensor_scalar_max(t_row[:], t_row[:], 0.0)
            if debug_taps:
                dma(dbg["dbg_t"].ap(), t_row[:])
            t_sb = row_T(t_row, 2, "t_sb")

            # qh partial [128, 8] = WQ2[:, tb2] @ t_c
            ps_qh = psx.tile([128, 8], f32, tag="px")
            for jm in range(8):
                for kc in range(2):
                    nc.tensor.matmul(ps_qh[:, jm:jm + 1],
                                     wq2s[:, kc * H2 + jm * 128:
                                          kc * H2 + jm * 128 + 128],
                                     t_sb[:, kc:kc + 1],
                                     start=(kc == 0), stop=(kc == 1))
            qh_p = cpool.tile([128, 8], f32, tag="qh_p")
            nc.vector.tensor_copy(qh_p[:], ps_qh[:])
            d_qh_i = dpool.tile([128, 8], f32, tag="d_qh_i")
            d_qh_o = dpool.tile([128, 8], f32, tag="d_qh_o")
            dma(d_qh_i[:], qh_p[:])
            nc.gpsimd.collective_compute("AllReduce", ALU.add, replica_groups=RG,
                                         ins=[d_qh_i.opt()], outs=[d_qh_o.opt()])
            nc.vector.tensor_add(enc_sb[:], enc_sb[:], ce_b2_sb[:])
            dma(out.ap()[10:2058].rearrange("(b p) -> p b", p=128), enc_sb[:])
            qh_sb = cpool.tile([128, 8], f32, tag="qh_sb")
            dma(qh_sb[:], d_qh_o[:])
            nc.vector.tensor_add(qh_sb[:], qh_sb[:], qbias_sb[:])
            nc.vector.tensor_scalar_mul(qh_sb[:], qh_sb[:], ISCALE)
            if debug_taps:
                dma(dbg["dbg_qh"].ap(), qh_sb[:])

            # masked per-stripe-pair stationaries (fp8, pre-scaled by S8).
            # DoubleRow layout: pair pj covers stripes j=2pj (slot i=0) and
            # j=2pj+1 (slot i=1); each slot is 16 cols (8 used + 8 pad).
            qkm = cpool.tile([128, 4 * 32], f8, tag="qkm")
            nc.vector.memset(qkm[:], 0.0)
            for j in range(8):
                pj, i = j // 2, j % 2
                dst = pj * 32 + i * 16 + j
                nc.vector.tensor_scalar_mul(qkm[:, dst:dst + 1],
                                            qh_sb[:, j:j + 1], S8)
            # c_h = bk . qh  (per-head scalar, already has ISCALE via qh)
            qb = cpool.tile([128, 8], f32, tag="qb")
            nc.vector.tensor_mul(qb[:], qh_sb[:], bk8_sb[:])
            ps_c = psx.tile([8, 1], f32, tag="px")
            nc.tensor.matmul(ps_c[:], qb[:], ones_t[:, 0:1], start=True, stop=True)
            c_sb = cpool.tile([8, 1], f32, tag="c_sb")
            nc.vector.tensor_copy(c_sb[:], ps_c[:])

            # ---- m1 stages A (enc) + P (proto) into one psum, staged to SBUF
            # (issued here; PE executes them while waiting on AR latencies)
            # proto block first (needs only enc)
            eb = cpool.tile([1, H2], f32, tag="eb")
            dma(eb[:], d_enc_o[0:1, 0:H2])
            nc.vector.tensor_add(eb[:], eb[:], ce_b2r_sb[:])
            pr_sb = cpool.tile([NPROTO, H2], f32, tag="protos")
            dma(pr_sb[:], protos.ap())
            dif = cpool.tile([NPROTO, H2], f32, tag="dif")
            for nb in range(2):
                ps_eb = pss.tile([NPROTO, 512], f32, tag="s")
                nc.tensor.matmul(ps_eb[:], ones_t[0:1, 0:NPROTO],
                                 eb[:, ts(nb, 512)], start=True, stop=True)
                nc.vector.tensor_tensor(out=dif[:, ts(nb, 512)],
                                        in0=pr_sb[:, ts(nb, 512)],
                                        in1=ps_eb[:], op=ALU.subtract)
            nc.vector.tensor_mul(dif[:], dif[:], dif[:])
            d2 = cpool.tile([NPROTO, 1], f32, tag="d2")
            nc.vector.tensor_reduce(out=d2[:], in_=dif[:], axis=AX.X, op=ALU.add)
            ptd = pstr.tile([1, 64], f32, tag="tr")
            nc.tensor.transpose(ptd[:], d2[:], ident[0:64, 0:64])
            dt_ = cpool.tile([1, 64], f32, tag="dt_")
            nc.vector.tensor_copy(dt_[:], ptd[:])
            dmin2 = cpool.tile([1, 1], f32, tag="dmin2")
            nc.vector.tensor_reduce(out=dmin2[:], in_=dt_[:], axis=AX.X, op=ALU.min)
            ps_dm = pstr.tile([NPROTO, 1], f32, tag="tr")
            nc.tensor.matmul(ps_dm[:], ones_t[0:1, 0:NPROTO], dmin2[:],
                             start=True, stop=True)
            oh64 = cpool.tile([NPROTO, 1], f32, tag="oh64")
            nc.vector.tensor_tensor(out=oh64[:], in0=d2[:],
                                    in1=ps_dm[:], op=ALU.is_equal)
            psel = cpool.tile([1, H2], f32, tag="psel")
            for nb in range(2):
                ps_ps = pss.tile([1, 512], f32, tag="s")
                nc.tensor.matmul(ps_ps[:], oh64[:],
                                 pr_sb[:, ts(nb, 512)], start=True, stop=True)
                nc.vector.tensor_copy(psel[:, ts(nb, 512)], ps_ps[:])
            dmin = cpool.tile([1, 1], f32, tag="dmin")
            nc.scalar.sqrt(dmin[:], dmin2[:])
            conf = cpool.tile([1, 1], f32, tag="conf")
            nc.vector.tensor_scalar_add(conf[:], dmin[:], 1.0)
            nc.vector.reciprocal(conf[:], conf[:])
            d_prow = dpool.tile([H2], f32, tag="d_prow")
            dma(d_prow[:].rearrange("(o b) -> o b", o=1), psel[:])
            ppad = cpool.tile([128, 16], f32, tag="ppad")
            nc.vector.memset(ppad[:], 0.0)
            dma(ppad[:, 0:8], d_prow[:].rearrange("(b p) -> p b", p=128))
            ppad_b = cpool.tile([128, 8], bf16, tag="ppad_b")
            nc.vector.tensor_copy(ppad_b[:], ppad[:, 0:8])
            dma(out.ap()[4106:6154].rearrange("(b p) -> p b", p=128), ppad[:])

            # m1 A + P partial
            ps_m1 = psx.tile([1, 256], f32, tag="px")
            for k in range(16):
                nc.tensor.matmul(ps_m1[:], enc_b[:, k:k + 1],
                                 mp1e_s[:, ts(k, 256)],
                                 start=(k == 0), stop=False)
            for k in range(8):
                nc.tensor.matmul(ps_m1[:], ppad_b[:, k:k + 1],
                                 mp1p_s[:, ts(k, 256)],
                                 start=False, stop=(k == 7))
            m1acc = cpool.tile([1, 256], f32, tag="m1acc")
            nc.vector.tensor_tensor(out=m1acc[:], in0=ps_m1[:],
                                    in1=mp_b1_sb[:], op=ALU.add)

            # ================= K-PASS =================
            # paired transposed weights: slot t2 covers m-tiles 2*t2, 2*t2+1
            wpair = cpool.tile([128, 25 * 32], f8, tag="wpair")
            nc.vector.memset(wpair[:], 0.0)
            zacc = cpool.tile([8, 16], f32, tag="zacc")
            nc.vector.memset(zacc[:], 0.0)
            for mc in range(NCH):
                cw = 512 if mc < 12 else 128
                ps_s = pss.tile([8, 512], f32, tag="s")
                khv = khs[:].rearrange("p (j m) -> p j m", m=MPAD)
                for pj in range(4):
                    nc.tensor.matmul(
                        ps_s[:, 0:cw],
                        qkm[:, pj * 32:(pj + 1) * 32]
                        .rearrange("p (i h) -> p i h", i=2)[:, :, 0:8],
                        khv[:, 2 * pj:2 * pj + 2, mc * 512:mc * 512 + cw],
                        start=(pj == 0), stop=(pj == 3),
                        perf_mode=mybir.MatmulPerfMode.DoubleRow)
                w_c = cpool.tile([8, 512], f32, tag="w_c")
                if mc < 12:
                    nc.scalar.activation(w_c[:, 0:cw], ps_s[:, 0:cw], ACTF.Exp,
                                         bias=c_sb[:], scale=1.0 / S8,
                                         accum_out=zacc[:, mc:mc + 1])
                else:
                    nc.scalar.activation(w_c[:, 0:cw], ps_s[:, 0:cw], ACTF.Exp,
                                         bias=c_sb[:], scale=1.0 / S8)
                    nc.vector.memset(w_c[:, MVALID_TAIL:cw], 0.0)
                    nc.vector.tensor_reduce(out=zacc[:, mc:mc + 1],
                                            in_=w_c[:, 0:cw], axis=AX.X,
                                            op=ALU.add)
                if debug_taps and mc == 0:
                    dma(dbg["dbg_w0"].ap(), w_c[:])
                for ti in range(cw // 128):
                    gt = mc * 4 + ti
                    pt = pstr.tile([128, 8], f32, tag="tr")
                    nc.tensor.transpose(pt[:], w_c[0:8, ts(ti, 128)],
                                        ident[0:8, 0:8])
                    dst = (gt // 2) * 32 + (gt % 2) * 16
                    nc.vector.tensor_copy(wpair[:, dst:dst + 8], pt[:])

            # local Z per head
            zloc = cpool.tile([8, 1], f32, tag="zloc")
            nc.vector.tensor_reduce(out=zloc[:], in_=zacc[:, 0:NCH], axis=AX.X,
                                    op=ALU.add)
            if debug_taps:
                dma(dbg["dbg_z"].ap(), zacc[:])

            # ---- top-5 candidates (overlaps V-pass) ----
            rz8 = cpool.tile([8, 1], f32, tag="rz8")
            nc.vector.reciprocal(rz8[:], zloc[:])
            nc.vector.tensor_scalar_mul(rz8[:], rz8[:], 1.0 / (NH * NCN))
            zq = col_rep(rz8[:], "zq")
            awf = cpool.tile([128, 50 * 8], f32, tag="awf")
            nc.vector.tensor_tensor(
                out=awf[:].rearrange("p (a b) -> p a b", b=8),
                in0=wpair[:].rearrange("p (a g b) -> p a g b", g=2, b=8)[:, :, 0, :],
                in1=_b3(zq, 50), op=ALU.mult)
            attnw = cpool.tile([128, 50], f32, tag="attnw")
            nc.vector.tensor_reduce(out=attnw[:],
                                    in_=awf[:].rearrange("p (a b) -> p a b", b=8),
                                    axis=AX.X, op=ALU.add)
            cand1 = cpool.tile([128, 8], f32, tag="cand1")
            nc.vector.max(out=cand1[:], in_=attnw[:])
            ptc1 = pstr.tile([8, 128], f32, tag="tr")
            nc.tensor.transpose(ptc1[:], cand1[:], ident[:, :])
            cd2 = cpool.tile([8, 128], f32, tag="cd2")
            nc.vector.tensor_copy(cd2[:], ptc1[:])
            cand2 = cpool.tile([8, 8], f32, tag="cand2")
            nc.vector.max(out=cand2[:], in_=cd2[:])
            d_c64 = dpool.tile([64], f32, tag="d_c64")
            dma(d_c64[:].rearrange("(p b) -> p b", b=8), cand2[:])
            c64 = cpool.tile([1, 64], f32, tag="c64")
            dma(c64[:], d_c64[:].rearrange("(o b) -> o b", o=1))
            top8 = cpool.tile([1, 8], f32, tag="top8")
            nc.vector.max(out=top8[:], in_=c64[:])
            slots = cpool.tile([1, 5 * NCN], f32, tag="slots")
            for i in range(NCN):
                nc.vector.tensor_copy(slots[:, i * 5:(i + 1) * 5], top8[:, 0:5])
            nc.vector.tensor_mul(slots[:], slots[:], slot_sb[:])

            # ================= V-PASS =================
            ps_u = []
            for nb in range(4):
                ps_unb = psu.tile([8, 512], f32, tag=f"u{nb}", name=f"ps_u{nb}")
                ps_u.append(ps_unb)
            for cd in range(NCH):
                ntile = 4 if cd < 12 else 1
                rows = 512 if cd < 12 else 128
                vt = vpool.tile([128, 4 * HID], f8, tag="vt")
                dma(vt[:, 0:ntile * HID].rearrange("p (mc d) -> p mc d", d=HID),
                    vh.ap()[cd * 512: cd * 512 + rows, :]
                    .rearrange("(mc p) d -> p mc d", p=128))
                vtv = vt[:].rearrange("p (mc d) -> p mc d", d=HID)
                if cd < 12:
                    for t2l in range(2):
                        t2 = cd * 2 + t2l
                        for nb in range(4):
                            nc.tensor.matmul(
                                ps_u[nb][:],
                                wpair[:, t2 * 32:(t2 + 1) * 32]
                                .rearrange("p (i h) -> p i h", i=2)[:, :, 0:8],
                                vtv[:, 2 * t2l:2 * t2l + 2,
                                    nb * 512:nb * 512 + 512],
                                start=(t2 == 0), stop=False,
                                perf_mode=mybir.MatmulPerfMode.DoubleRow)
                else:
                    for nb in range(4):
                        nc.tensor.matmul(ps_u[nb][:],
                                         wpair[:, 24 * 32:24 * 32 + 8],
                                         vt[:, nb * 512:nb * 512 + 512],
                                         start=False, stop=True)

            # ---- u AllReduce payload: [8, 2048 u | 1 Z | 40 slots | 7 pad]
            UW = 3896
            d_u_i = dpool.tile([8, UW], bf16, tag="d_u_i")
            d_u_o = dpool.tile([8, UW], bf16, tag="d_u_o")
            u_s = cpool.tile([8, HID], bf16, tag="u_s")
            for nb in range(4):
                nc.vector.tensor_copy(u_s[:, ts(nb, 512)], ps_u[nb][:])
            dma(d_u_i[:].rearrange("h w -> (h w)")[ds(1792, 8 * (UW - 256))]
                .rearrange("(h d) -> h d", d=UW - 256)[:, 0:HID],
                u_s[:])
            stg = cpool.tile([8, 48], bf16, tag="stg")
            nc.vector.memset(stg[:], 0.0)
            nc.vector.tensor_copy(stg[:, 0:1], zloc[:])
            nc.vector.tensor_copy(stg[0:1, 1:41], slots[:])
            dma(d_u_i[:, 3840:3888], stg[:])
            nc.gpsimd.collective_compute("AllReduce", ALU.add, replica_groups=RG,
                                         ins=[d_u_i.opt()], outs=[d_u_o.opt()])

            # ---- post-AR: Z, top5, ctx extraction ----
            G = cpool.tile([8, 48], bf16, tag="G")
            dma(G[:], d_u_o[:, 3840:3888])
            zg = cpool.tile([8, 1], f32, tag="zg")
            nc.vector.reciprocal(zg[:], G[:, 0:1])
            top40 = cpool.tile([1, 5 * NCN], f32, tag="top40")
            nc.vector.tensor_copy(top40[:], G[0:1, 1:41])
            top8f = cpool.tile([1, 8], f32, tag="top8f")
            nc.vector.max(out=top8f[:], in_=top40[:])

            ctxm_b = cpool.tile([8, 256], bf16, tag="ctxm_b")
            dma(ctxm_b[:], d_u_o[:, 1792:2048])
            ctxm = cpool.tile([8, 256], f32, tag="ctxm")
            nc.vector.tensor_scalar(out=ctxm[:], in0=ctxm_b[:], scalar1=zg[:],
                                    scalar2=None, op0=ALU.mult)
            if debug_taps:
                dma(dbg["dbg_u"].ap(),
                    d_u_o[:].rearrange("h w -> (h w)")[ds(1792, 8 * (UW - 256))]
                    .rearrange("(h d) -> h d", d=UW - 256)[:, 0:HID])
            ps_cr = psx.tile([1, 256], f32, tag="px")
            nc.tensor.matmul(ps_cr[:], oh8_sb[:], ctxm[:], start=True, stop=True)
            ctx_row = cpool.tile([1, 256], f32, tag="ctx_row")
            nc.vector.tensor_copy(ctx_row[:], ps_cr[:])
            ctx_sb = row_T(ctx_row, 2, "ctx_sb")
            # full ctx in p-major layout for the folded m1-attended stage:
            # col k holds ctx[k*128 : (k+1)*128] = ctxm[k//2, (k%2)*128 : +128]
            ctx_pm = cpool.tile([128, 16], bf16, tag="ctx_pm")
            ctxT = cpool.tile([128, 16], f32, tag="ctxT")
            for j in range(2):
                ptc = pstr.tile([128, 8], f32, tag="tr")
                nc.tensor.transpose(ptc[:], ctxm[0:8, j * 128:(j + 1) * 128],
                                    ident[0:8, 0:8])
                nc.vector.tensor_copy(ctxT[:, ts(j, 8)], ptc[:])
            for k in range(16):
                nc.vector.tensor_copy(ctx_pm[:, k:k + 1],
                                      ctxT[:, (k % 2) * 8 + k // 2:
                                           (k % 2) * 8 + k // 2 + 1])
            if debug_taps:
                dma(dbg["dbg_ctx"].ap(), ctx_sb[:])

            # attended partial = wo[:, b2] @ ctx_c   (row orientation)
            att_p = cpool.tile([1, HID], f32, tag="rowst", name="att_p")
            for nb in range(4):
                ps_at = pss.tile([1, 512], f32, tag="s")
                for kc in range(2):
                    nc.tensor.matmul(ps_at[:], ctx_sb[:, kc:kc + 1],
                                     woS[:, kc * HID + nb * 512:
                                         kc * HID + nb * 512 + 512],
                                     start=(kc == 0), stop=(kc == 1))
                nc.vector.tensor_copy(att_p[:, ts(nb, 512)], ps_at[:])
            d_a_i = dpool.tile([1, HID], f32, tag="d_a_i")
            d_a_o = dpool.tile([1, HID], f32, tag="d_a_o")
            dma(d_a_i[:], att_p[:])
            # m1 stage B from folded (mp1a @ wo) @ ctx — independent of att-AR
            ps_m1b = psx.tile([1, 256], f32, tag="px")
            for k in range(16):
                nc.tensor.matmul(ps_m1b[:], ctx_pm[:, k:k + 1],
                                 mp1a_s[:, ts(k, 256)],
                                 start=(k == 0), stop=(k == 15))
            m1_row = cpool.tile([1, 256], f32, tag="m1_row")
            nc.vector.tensor_add(m1_row[:], ps_m1b[:], m1acc[:])
            nc.vector.tensor_scalar_max(m1_row[:], m1_row[:], 0.0)
            nc.vector.tensor_mul(m1_row[:], m1_row[:], bn2sc_sb[:])
            nc.vector.tensor_add(m1_row[:], m1_row[:], bn2sh_sb[:])
            if debug_taps:
                dma(dbg["dbg_m1"].ap(), m1_row[:])
            m1_sb = row_T(m1_row, 2, "m1_sb")

            # m2 partial = mp_w2[:, b2] @ m1_c   (row orientation)
            m2_pf = cpool.tile([1, HID], f32, tag="rowst", name="m2_pf")
            m2_p = m2_pf[0:1, 0:H2]
            for nb in range(2):
                ps_m2 = pss.tile([1, 512], f32, tag="s")
                for kc in range(2):
                    nc.tensor.matmul(ps_m2[:], m1_sb[:, kc:kc + 1],
                                     mp_w2s[:, kc * H2 + nb * 512:
                                            kc * H2 + nb * 512 + 512],
                                     start=(kc == 0), stop=(kc == 1))
                nc.vector.tensor_copy(m2_p[:, ts(nb, 512)], ps_m2[:])
            d_m2_i = dpool.tile([1, H2], f32, tag="d_m2_i")
            d_m2_o = dpool.tile([1, H2], f32, tag="d_m2_o")
            dma(d_m2_i[:], m2_p)
            nc.gpsimd.collective_compute("AllReduce", ALU.add, replica_groups=RG,
                                         ins=[d_m2_i.opt()], outs=[d_m2_o.opt()])
            nc.gpsimd.collective_compute("AllReduce", ALU.add, replica_groups=RG,
                                         ins=[d_a_i.opt()], outs=[d_a_o.opt()])
            m2_sb = cpool.tile([128, 8], f32, tag="m2_sb")
            dma(m2_sb[:], d_m2_o[:].rearrange("o (k p) -> (o p) k", p=128))
            nc.vector.tensor_add(m2_sb[:], m2_sb[:], mp_b2_sb[:])
            nc.vector.tensor_scalar_max(m2_sb[:], m2_sb[:], 0.0)
            m2_b = cpool.tile([128, 8], bf16, tag="m2_b")
            nc.vector.tensor_copy(m2_b[:], m2_sb[:])

            # meta = mp_w3 @ m2 + b3
            ps_mt = psx.tile([4, 1], f32, tag="px")
            for k in range(8):
                nc.tensor.matmul(ps_mt[:], mp_w3s[:, ts(k, 4)],
                                 m2_b[:, k:k + 1],
                                 start=(k == 0), stop=(k == 7))
            meta_sb = cpool.tile([4, 1], f32, tag="meta_sb")
            nc.vector.tensor_add(meta_sb[:], ps_mt[:], mp_b3_sb[:])
            ptmt = pstr.tile([1, 4], f32, tag="tr")
            nc.tensor.transpose(ptmt[:], meta_sb[:], ident[0:4, 0:4])
            metaT = cpool.tile([1, 4], f32, tag="metaT")
            nc.vector.tensor_copy(metaT[:], ptmt[:])
            nmax = cpool.tile([1, 1], f32, tag="nmax")
            nc.vector.tensor_reduce(out=nmax[:], in_=metaT[:, 0:3], axis=AX.X,
                                    op=ALU.max)
            nc.vector.tensor_scalar_mul(nmax[:], nmax[:], -1.0)
            e3 = cpool.tile([1, 3], f32, tag="e3")
            nc.scalar.activation(e3[:], metaT[:, 0:3], ACTF.Exp, bias=nmax[:])
            s3 = cpool.tile([1, 1], f32, tag="s3")
            nc.vector.tensor_reduce(out=s3[:], in_=e3[:], axis=AX.X, op=ALU.add)
            nc.vector.reciprocal(s3[:], s3[:])
            regime = cpool.tile([1, 3], f32, tag="regime")
            nc.vector.tensor_scalar(out=regime[:], in0=e3[:], scalar1=s3[:],
                                    scalar2=None, op0=ALU.mult)
            crisis = cpool.tile([1, 1], f32, tag="crisis")
            nc.scalar.activation(crisis[:], metaT[:, 3:4], ACTF.Sigmoid)

            att_f = cpool.tile([128, 16], f32, tag="att_f")
            dma(att_f[:], d_a_o[:].rearrange("o (k p) -> (o p) k", p=128))
            nc.vector.tensor_add(att_f[:], att_f[:], bob_sb[:])
            dma(out.ap()[2058:4106].rearrange("(b p) -> p b", p=128), att_f[:])

            # ---- output assembly ----
            dma(out.ap()[0:3].rearrange("(o b) -> o b", o=1), regime[:])
            dma(out.ap()[3:4].rearrange("(o b) -> o b", o=1), crisis[:])
            dma(out.ap()[4:5].rearrange("(o b) -> o b", o=1), conf[:])
            dma(out.ap()[5:10].rearrange("(o b) -> o b", o=1), top8f[:, 0:5])

    nc.compile()
    return nc


_NC_CACHE = {}


def _get_nc():
    if "nc" not in _NC_CACHE:
        _NC_CACHE["nc"] = build_nc()
    return _NC_CACHE["nc"]


def _bm(x, nb):
    """vector [nb*128] -> b-major [128, nb] (col b = x[b*128:(b+1)*128])."""
    return np.ascontiguousarray(np.asarray(x, np.float32).reshape(nb, 128).T)


def _bf(x):
    return np.ascontiguousarray(np.asarray(x)).astype(ml_dtypes.bfloat16)


def _f8(x):
    return np.ascontiguousarray(np.asarray(x)).astype(ml_dtypes.float8_e4m3)


def shard_inputs(i):
    g = {k: np.asarray(v, np.float32) for k, v in i.items()}
    # host folds
    kh = g["memory_keys"] @ g["wk"].T                       # [MEM, H2]
    vhf = g["memory_values"] @ g["wv"].T                    # [MEM, HID]
    WQ2 = g["wq"] @ g["qe_w2"]                              # [H2, HID]
    qbias_full = g["wq"] @ g["qe_b2"] + g["bq"]             # [H2]
    bob_full = g["wo"] @ g["bv"] + g["bo"]                  # [HID]
    bn1_scale = g["bn1_g"] / np.sqrt(g["bn1_v"] + EPS)
    bn1_shift = g["bn1_b"] - g["bn1_m"] * bn1_scale
    bn2_scale = g["bn2_g"] / np.sqrt(g["bn2_v"] + EPS)
    bn2_shift = g["bn2_b"] - g["bn2_m"] * bn2_scale

    in_maps = []
    for c in range(NCN):
        b2 = slice(c * 256, (c + 1) * 256)
        khp = np.zeros((MPAD, H2), np.float32)
        khp[0:MSH] = kh[c * MSH:(c + 1) * MSH]
        vhp = np.zeros((MPAD, HID), np.float32)
        vhp[0:MSH] = vhf[c * MSH:(c + 1) * MSH]
        oh = np.zeros((8, 1), np.float32); oh[c, 0] = 1.0
        sm = np.zeros((1, 40), np.float32); sm[0, c * 5:(c + 1) * 5] = 1.0
        m = {
            "obs": _bf(_bm(g["observation"], 32)),
            "w1T": _bf(g["ce_w1"][b2].T),
            "ce_b1r": g["ce_b1"][b2].reshape(1, 256),
            "bn1_sc": bn1_scale[b2].reshape(1, 256),
            "bn1_sh": bn1_shift[b2].reshape(1, 256),
            "ce_w2T": _bf(g["ce_w2"][:, b2].T),
            "ce_b2": _bm(g["ce_b2"], 16),
            "ce_b2r": g["ce_b2"][0:H2].reshape(1, H2),
            "qe_w1T": _bf(g["qe_w1"][b2].T),
            "qe_b1r": g["qe_b1"][b2].reshape(1, 256),
            "wq2T": _bf(WQ2[:, b2].T),
            "qbias": _bm(qbias_full, 8),
            "bk8": _bm(g["bk"], 8),
            "khT": _f8(khp.T),
            "vh": _f8(vhp),
            "woT": _bf(g["wo"][:, b2].T),
            "bob": _bm(bob_full, 16),
            "protos": np.ascontiguousarray(g["prototypes"]),
            "mp1eT": _bf(g["mp_w1"][b2, 0:HID].T),
            "mp1aT": _bf((g["mp_w1"][b2, HID:2 * HID] @ g["wo"]).T),
            "mp1pT": _bf(g["mp_w1"][b2, 2 * HID:2 * HID + H2].T),
            "mp_b1r": (g["mp_b1"][b2]
                       + g["mp_w1"][b2, HID:2 * HID] @ bob_full
                       ).reshape(1, 256),
            "bn2_sc": bn2_scale[b2].reshape(1, 256),
            "bn2_sh": bn2_shift[b2].reshape(1, 256),
            "mp_w2T": _bf(g["mp_w2"][:, b2].T),
            "mp_b2_8": _bm(g["mp_b2"], 8),
            "mp_w3T": _bf(g["mp_w3"].T),
            "mp_b3": np.asarray(g["mp_b3"], np.float32).reshape(4, 1).copy(),
            "oh8": oh,
            "slot_mask": sm,
        }
        in_maps.append(m)
    return in_maps


def kernel(**inputs):
    nc = _get_nc()
    in_maps = shard_inputs(inputs)
    res = bass_utils.run_bass_kernel_spmd(nc, in_maps, core_ids=list(range(NCN)))
    return np.asarray(res.results[0]["out"], np.float32)


# revision 24
# speedup vs baseline: 1.0143x; 1.0070x over previous
"""Trainium2 Bass kernel for nn_MetaLearningCrisisMemory (retrieval_knn).

Self-contained: kernel(**inputs) -> np.ndarray [6154] fp32.

v2 strategy (8-way SPMD, memory-bound target):
 - Host-fold wk into K (kh = K @ wk.T) and wv into V (vh = V @ wv.T): the
   two big device passes become pure streamed sweeps. kh/vh shipped fp8
   (e4m3); output-norm analysis shows the attended section carries ~0.07%
   of output norm^2, so fp8 noise there is negligible.
 - Scores bounded (~|1.3|): exp without max-subtraction; softmax
   normalization Z rides along the u-AllReduce. No flash-max machinery.
 - Matmuls in vector-stationary orientation with N=512 moving columns:
   ~500 PE instructions total (vs 3225 in v1 at a fixed ~213ns each).
 - 5 AllReduces: enc, qh, u(+Z+top5 slots), attended, m2.
 - All small Linears tensor-parallel with bf16 host-pre-transposed shards.
"""

import numpy as np
import ml_dtypes

import concourse.bass as bass
import concourse.mybir as mybir
import concourse.tile as tile
from concourse import bacc, bass_utils
from concourse.bass import ts, ds
from concourse.masks import make_identity

f32 = mybir.dt.float32
bf16 = mybir.dt.bfloat16
f8 = mybir.dt.float8e4
AX = mybir.AxisListType
ALU = mybir.AluOpType
ACTF = mybir.ActivationFunctionType

NCN = 8
INPUT_DIM, HID, MEM, NPROTO = 4096, 2048, 50000, 64
H2 = HID // 2                  # 1024
NH = 8
DQ = H2 // NH                  # 128
DV = HID // NH                 # 256
TOPK = 5
EPS = 1e-5
MSH = MEM // NCN               # 6250 rows per core
MPAD = 6272                    # padded to 49 * 128
MT = MPAD // 128               # 49 m-tiles
NCH = 13                       # 12 chunks of 512 + 1 of 128
MVALID_TAIL = 106              # valid rows in tile 48 (6250 - 48*128)
OUT_N = 3 + 1 + 1 + TOPK + 3 * HID  # 6154
ISCALE = 1.0 / float(np.sqrt(np.float32(DQ)))
S8 = 32.0                      # fp8 pre-scale for the query


def _din(nc, name, shape, dt=f32):
    return nc.dram_tensor(name, list(shape), dt, kind="ExternalInput")


def build_nc(debug_taps=False):
    nc = bacc.Bacc("TRN2", target_bir_lowering=False, debug=False,
                   enable_asserts=False, num_devices=NCN)

    # ---- I/O ----
    obs = _din(nc, "obs", (128, 32), bf16)
    w1T = _din(nc, "w1T", (INPUT_DIM, 256), bf16)
    ce_b1r = _din(nc, "ce_b1r", (1, 256))
    bn1_sc = _din(nc, "bn1_sc", (1, 256))
    bn1_sh = _din(nc, "bn1_sh", (1, 256))
    ce_w2T = _din(nc, "ce_w2T", (256, HID), bf16)
    encbr = _din(nc, "encbr", (1, HID))
    qe_w1T = _din(nc, "qe_w1T", (HID, 256), bf16)
    qe_b1r = _din(nc, "qe_b1r", (1, 256))
    wq2T = _din(nc, "wq2T", (256, H2), bf16)
    qbias = _din(nc, "qbias", (128, 8))
    bk8 = _din(nc, "bk8", (128, 8))
    khT = _din(nc, "khT", (H2, MPAD), f8)
    vh = _din(nc, "vh", (MPAD, HID), f8)
    woT = _din(nc, "woT", (256, HID), bf16)
    bob = _din(nc, "bob", (128, 16))
    protos = _din(nc, "protos", (NPROTO, H2))
    mp1eT = _din(nc, "mp1eT", (HID, 256), bf16)
    mp1aT = _din(nc, "mp1aT", (HID, 256), bf16)
    mp1pT = _din(nc, "mp1pT", (H2, 256), bf16)
    mp_b1r = _din(nc, "mp_b1r", (1, 256))
    bn2_sc = _din(nc, "bn2_sc", (1, 256))
    bn2_sh = _din(nc, "bn2_sh", (1, 256))
    mp_w2T = _din(nc, "mp_w2T", (256, H2), bf16)
    mp_b2_8 = _din(nc, "mp_b2_8", (128, 8))
    mp_w3T = _din(nc, "mp_w3T", (H2, 4), bf16)
    mp_b3 = _din(nc, "mp_b3", (4, 1))
    oh8 = _din(nc, "oh8", (8, 1))
    slot_mask = _din(nc, "slot_mask", (1, 5 * NCN))
    out = nc.dram_tensor("out", [OUT_N], f32, kind="ExternalOutput")
    dbg = {}
    if debug_taps:
        for nm, shp in (("dbg_w0", [8, 512]), ("dbg_qh", [128, 8]),
                        ("dbg_u", [8, HID]), ("dbg_ctx", [128, 2]),
                        ("dbg_z", [8, 16]), ("dbg_m1", [1, 256]),
                        ("dbg_h", [1, 256]), ("dbg_t", [1, 256])):
            dbg[nm] = nc.dram_tensor(nm, shp, f32, kind="ExternalOutput")

    RG = [list(range(NCN))]

    with tile.TileContext(nc) as tc:
        import contextlib
        with contextlib.ExitStack() as stk:
            cpool = stk.enter_context(tc.tile_pool(name="cpool", bufs=1))
            vpool = stk.enter_context(tc.tile_pool(name="vpool", bufs=3))
            psx = stk.enter_context(tc.tile_pool(name="psx", bufs=1, space="PSUM"))
            pss = stk.enter_context(tc.tile_pool(name="pss", bufs=2, space="PSUM"))
            pstr = stk.enter_context(tc.tile_pool(name="pstr", bufs=1, space="PSUM"))
            psu = stk.enter_context(tc.tile_pool(name="psu", bufs=1, space="PSUM"))
            dpool = stk.enter_context(tc.tile_pool(name="dpool", bufs=1, space="DRAM"))

            def dma(dst, src):
                nc.sync.dma_start(out=dst, in_=src)

            def load(shape, dram_t, tag, dt=f32):
                t = cpool.tile(list(shape), dt, tag=tag)
                dma(t[:], dram_t.ap())
                return t

            # ---- CC warm-up first: pulls the CC entry barrier to t~0 and
            # absorbs the cold-stream cost under the front compute
            d_w_i = dpool.tile([1, 8], f32, tag="d_w_i")
            d_w_o = dpool.tile([1, 8], f32, tag="d_w_o")
            nc.gpsimd.collective_compute("AllReduce", ALU.add, replica_groups=RG,
                                         ins=[d_w_i.opt()], outs=[d_w_o.opt()])

            # ---- constants ----
            ident = cpool.tile([128, 128], f32, tag="ident")
            make_identity(nc, ident[:])
            ones_t = cpool.tile([128, 128], f32, tag="ones_t")
            nc.vector.memset(ones_t[:], 1.0)

            def col_rep(col8, tagn):
                """[8,1] column -> [128,8] partition-replicated row values."""
                dg = cpool.tile([8, 8], f32, tag=tagn + "_dg")
                nc.vector.tensor_tensor(out=dg[:], in0=ident[0:8, 0:8],
                                        in1=col8.to_broadcast([8, 8]), op=ALU.mult)
                pr = pstr.tile([128, 8], f32, tag="tr")
                nc.tensor.matmul(pr[:], ones_t[0:8, :], dg[:], start=True, stop=True)
                rep = cpool.tile([128, 8], f32, tag=tagn)
                nc.vector.tensor_copy(rep[:], pr[:])
                return rep

            def _b3(rep, nrep):
                return rep[:].unsqueeze(1).broadcast_to([128, nrep, 8])

            def row_T(row_ap, n128, tagout, dt=bf16):
                """[1, n128*128] fp32 row -> [128, n128] tile (dtype dt)."""
                o = cpool.tile([128, n128], dt, tag=tagout)
                for k in range(n128):
                    pt = pstr.tile([128, 1], f32, tag="tr")
                    nc.tensor.transpose(pt[:], row_ap[0:1, ts(k, 128)],
                                        ident[0:1, 0:1])
                    nc.vector.tensor_copy(o[:, k:k + 1], pt[:])
                return o

            # ---- big streaming loads (issued early) ----
            obs_sb = load((128, 32), obs, "obs", bf16)
            w1s = cpool.tile([128, 32 * 256], bf16, tag="w1s")
            dma(w1s[:].rearrange("p (k m) -> p k m", m=256),
                w1T.ap().rearrange("(k p) m -> p k m", p=128))
            ce_w2s = cpool.tile([128, 2 * HID], bf16, tag="ce_w2s")
            dma(ce_w2s[:].rearrange("p (k m) -> p k m", m=HID),
                ce_w2T.ap().rearrange("(k p) m -> p k m", p=128))
            qe_w1s = cpool.tile([128, 16 * 256], bf16, tag="qe_w1s")
            dma(qe_w1s[:].rearrange("p (k m) -> p k m", m=256),
                qe_w1T.ap().rearrange("(k p) m -> p k m", p=128))
            wq2s = cpool.tile([128, 2 * H2], bf16, tag="wq2s")
            dma(wq2s[:].rearrange("p (k m) -> p k m", m=H2),
                wq2T.ap().rearrange("(k p) m -> p k m", p=128))

            ce_b1_sb = load((1, 256), ce_b1r, "ce_b1")
            bn1sc_sb = load((1, 256), bn1_sc, "bn1sc")
            bn1sh_sb = load((1, 256), bn1_sh, "bn1sh")
            encbr_sb = load((1, HID), encbr, "encbr")
            qe_b1_sb = load((1, 256), qe_b1r, "qe_b1")
            qbias_sb = load((128, 8), qbias, "qbias")
            bk8_sb = load((128, 8), bk8, "bk8")
            bob_sb = load((128, 16), bob, "bob")
            mp_b1_sb = load((1, 256), mp_b1r, "mp_b1")
            bn2sc_sb = load((1, 256), bn2_sc, "bn2sc")
            bn2sh_sb = load((1, 256), bn2_sh, "bn2sh")
            mp_b2_sb = load((128, 8), mp_b2_8, "mp_b2")
            mp_b3_sb = load((4, 1), mp_b3, "mp_b3")
            oh8_sb = load((8, 1), oh8, "oh8")
            slot_sb = load((1, 5 * NCN), slot_mask, "slot")

            khs = cpool.tile([128, 8 * MPAD], f8, tag="khs")
            dma(khs[:].rearrange("p (j m) -> p j m", m=MPAD),
                khT.ap().rearrange("(j p) m -> p j m", p=128))
            woS = cpool.tile([128, 2 * HID], bf16, tag="woS")
            dma(woS[:].rearrange("p (k m) -> p k m", m=HID),
                woT.ap().rearrange("(k p) m -> p k m", p=128))
            mp1e_s = cpool.tile([128, 16 * 256], bf16, tag="mp1e_s")
            dma(mp1e_s[:].rearrange("p (k m) -> p k m", m=256),
                mp1eT.ap().rearrange("(k p) m -> p k m", p=128))
            mp1a_s = cpool.tile([128, 16 * 256], bf16, tag="mp1a_s")
            dma(mp1a_s[:].rearrange("p (k m) -> p k m", m=256),
                mp1aT.ap().rearrange("(k p) m -> p k m", p=128))
            mp1p_s = cpool.tile([128, 8 * 256], bf16, tag="mp1p_s")
            dma(mp1p_s[:].rearrange("p (k m) -> p k m", m=256),
                mp1pT.ap().rearrange("(k p) m -> p k m", p=128))
            mp_w2s = cpool.tile([128, 2 * H2], bf16, tag="mp_w2s")
            dma(mp_w2s[:].rearrange("p (k m) -> p k m", m=H2),
                mp_w2T.ap().rearrange("(k p) m -> p k m", p=128))
            mp_w3s = cpool.tile([128, 8 * 4], bf16, tag="mp_w3s")
            dma(mp_w3s[:].rearrange("p (k m) -> p k m", m=4),
                mp_w3T.ap().rearrange("(k p) m -> p k m", p=128))

            # ================= FRONT =================
            # L1: h_row = bn1(relu(ce_w1[b2] @ obs + b1))   [1, 256]
            ps_h = psx.tile([1, 256], f32, tag="px")
            for k in range(32):
                nc.tensor.matmul(ps_h[:], obs_sb[:, k:k + 1],
                                 w1s[:, ts(k, 256)],
                                 start=(k == 0), stop=(k == 31))
            h_row = cpool.tile([1, 256], f32, tag="h_row")
            nc.vector.tensor_add(h_row[:], ps_h[:], ce_b1_sb[:])
            nc.vector.tensor_scalar_max(h_row[:], h_row[:], 0.0)
            nc.vector.tensor_mul(h_row[:], h_row[:], bn1sc_sb[:])
            nc.vector.tensor_add(h_row[:], h_row[:], bn1sh_sb[:])
            if debug_taps:
                dma(dbg["dbg_h"].ap(), h_row[:])
            h_sb = row_T(h_row, 2, "h_sb")

            # L2: enc partial [1, 2048] = ce_w2[:, b2] @ h_c
            enc_p = cpool.tile([1, HID], f32, tag="rowst", name="enc_p")
            for nb in range(4):
                ps_e = pss.tile([1, 512], f32, tag="s")
                for kc in range(2):
                    nc.tensor.matmul(ps_e[:], h_sb[:, kc:kc + 1],
                                     ce_w2s[:, kc * HID + nb * 512:
                                            kc * HID + nb * 512 + 512],
                                     start=(kc == 0), stop=(kc == 1))
                nc.vector.tensor_copy(enc_p[:, ts(nb, 512)], ps_e[:])
            nc.vector.tensor_add(enc_p[:], enc_p[:], encbr_sb[:])
            d_enc_i = dpool.tile([1, HID], f32, tag="d_enc_i")
            d_enc_o = dpool.tile([1, HID], f32, tag="d_enc_o")
            dma(d_enc_i[:], enc_p[:])
            nc.gpsimd.collective_compute("AllReduce", ALU.add, replica_groups=RG,
                                         ins=[d_enc_i.opt()], outs=[d_enc_o.opt()])
            enc_sb = cpool.tile([128, 16], f32, tag="enc_sb")
            dma(enc_sb[:], d_enc_o[:].rearrange("o (k p) -> (o p) k", p=128))
            enc_b = cpool.tile([128, 16], bf16, tag="enc_b")
            nc.vector.tensor_copy(enc_b[:], enc_sb[:])

            # query path: t = relu(qe_w1[b2] @ enc + b)    [1, 256]
            ps_t = psx.tile([1, 256], f32, tag="px")
            for k in range(16):
                nc.tensor.matmul(ps_t[:], enc_b[:, k:k + 1],
                                 qe_w1s[:, ts(k, 256)],
                                 start=(k == 0), stop=(k == 15))
            t_row = cpool.tile([1, 256], f32, tag="t_row")
            nc.vector.tensor_add(t_row[:], ps_t[:], qe_b1_sb[:])
            nc.vector.tensor_scalar_max(t_row[:], t_row[:], 0.0)
            if debug_taps:
                dma(dbg["dbg_t"].ap(), t_row[:])
            t_sb = row_T(t_row, 2, "t_sb")

            # qh partial [128, 8] = WQ2[:, tb2] @ t_c
            ps_qh = psx.tile([128, 8], f32, tag="px")
            for jm in range(8):
                for kc in range(2):
                    nc.tensor.matmul(ps_qh[:, jm:jm + 1],
                                     wq2s[:, kc * H2 + jm * 128:
                                          kc * H2 + jm * 128 + 128],
                                     t_sb[:, kc:kc + 1],
                                     start=(kc == 0), stop=(kc == 1))
            qh_p = cpool.tile([128, 8], f32, tag="qh_p")
            nc.vector.tensor_tensor(out=qh_p[:], in0=ps_qh[:],
                                    in1=qbias_sb[:], op=ALU.add)
            d_qh_i = dpool.tile([128, 8], f32, tag="d_qh_i")
            d_qh_o = dpool.tile([128, 8], f32, tag="d_qh_o")
            dma(d_qh_i[:], qh_p[:])
            nc.gpsimd.collective_compute("AllReduce", ALU.add, replica_groups=RG,
                                         ins=[d_qh_i.opt()], outs=[d_qh_o.opt()])
            dma(out.ap()[10:2058].rearrange("(b p) -> p b", p=128), enc_sb[:])
            qh_sb = cpool.tile([128, 8], f32, tag="qh_sb")
            dma(qh_sb[:], d_qh_o[:])
            if debug_taps:
                dma(dbg["dbg_qh"].ap(), qh_sb[:])

            # masked per-stripe-pair stationaries (fp8, pre-scaled by S8).
            # DoubleRow layout: pair pj covers stripes j=2pj (slot i=0) and
            # j=2pj+1 (slot i=1); each slot is 16 cols (8 used + 8 pad).
            qkm = cpool.tile([128, 4 * 32], f8, tag="qkm")
            nc.vector.memset(qkm[:], 0.0)
            for j in range(8):
                pj, i = j // 2, j % 2
                dst = pj * 32 + i * 16 + j
                nc.vector.tensor_scalar_mul(qkm[:, dst:dst + 1],
                                            qh_sb[:, j:j + 1], S8)
            # c_h = bk . qh  (per-head scalar, already has ISCALE via qh)
            qb = cpool.tile([128, 8], f32, tag="qb")
            nc.vector.tensor_mul(qb[:], qh_sb[:], bk8_sb[:])
            ps_c = psx.tile([8, 1], f32, tag="px")
            nc.tensor.matmul(ps_c[:], qb[:], ones_t[:, 0:1], start=True, stop=True)
            c_sb = cpool.tile([8, 1], f32, tag="c_sb")
            nc.vector.tensor_copy(c_sb[:], ps_c[:])

            # ---- m1 stages A (enc) + P (proto) into one psum, staged to SBUF
            # (issued here; PE executes them while waiting on AR latencies)
            # proto block first (needs only enc)
            eb = cpool.tile([1, H2], f32, tag="eb")
            dma(eb[:], d_enc_o[0:1, 0:H2])
            pr_sb = cpool.tile([NPROTO, H2], f32, tag="protos")
            dma(pr_sb[:], protos.ap())
            dif = cpool.tile([NPROTO, H2], f32, tag="dif")
            for nb in range(2):
                ps_eb = pss.tile([NPROTO, 512], f32, tag="s")
                nc.tensor.matmul(ps_eb[:], ones_t[0:1, 0:NPROTO],
                                 eb[:, ts(nb, 512)], start=True, stop=True)
                nc.vector.tensor_tensor(out=dif[:, ts(nb, 512)],
                                        in0=pr_sb[:, ts(nb, 512)],
                                        in1=ps_eb[:], op=ALU.subtract)
            nc.vector.tensor_mul(dif[:], dif[:], dif[:])
            d2 = cpool.tile([NPROTO, 1], f32, tag="d2")
            nc.vector.tensor_reduce(out=d2[:], in_=dif[:], axis=AX.X, op=ALU.add)
            ptd = pstr.tile([1, 64], f32, tag="tr")
            nc.tensor.transpose(ptd[:], d2[:], ident[0:64, 0:64])
            dt_ = cpool.tile([1, 64], f32, tag="dt_")
            nc.vector.tensor_copy(dt_[:], ptd[:])
            dmin2 = cpool.tile([1, 1], f32, tag="dmin2")
            nc.vector.tensor_reduce(out=dmin2[:], in_=dt_[:], axis=AX.X, op=ALU.min)
            ps_dm = pstr.tile([NPROTO, 1], f32, tag="tr")
            nc.tensor.matmul(ps_dm[:], ones_t[0:1, 0:NPROTO], dmin2[:],
                             start=True, stop=True)
            oh64 = cpool.tile([NPROTO, 1], f32, tag="oh64")
            nc.vector.tensor_tensor(out=oh64[:], in0=d2[:],
                                    in1=ps_dm[:], op=ALU.is_equal)
            psel = cpool.tile([1, H2], f32, tag="psel")
            for nb in range(2):
                ps_ps = pss.tile([1, 512], f32, tag="s")
                nc.tensor.matmul(ps_ps[:], oh64[:],
                                 pr_sb[:, ts(nb, 512)], start=True, stop=True)
                nc.vector.tensor_copy(psel[:, ts(nb, 512)], ps_ps[:])
            dmin = cpool.tile([1, 1], f32, tag="dmin")
            nc.scalar.sqrt(dmin[:], dmin2[:])
            conf = cpool.tile([1, 1], f32, tag="conf")
            nc.vector.tensor_scalar_add(conf[:], dmin[:], 1.0)
            nc.vector.reciprocal(conf[:], conf[:])
            d_prow = dpool.tile([H2], f32, tag="d_prow")
            dma(d_prow[:].rearrange("(o b) -> o b", o=1), psel[:])
            ppad = cpool.tile([128, 16], f32, tag="ppad")
            nc.vector.memset(ppad[:], 0.0)
            dma(ppad[:, 0:8], d_prow[:].rearrange("(b p) -> p b", p=128))
            ppad_b = cpool.tile([128, 8], bf16, tag="ppad_b")
            nc.vector.tensor_copy(ppad_b[:], ppad[:, 0:8])
            dma(out.ap()[4106:6154].rearrange("(b p) -> p b", p=128), ppad[:])

            # m1 A + P partial
            ps_m1 = psx.tile([1, 256], f32, tag="px")
            for k in range(16):
                nc.tensor.matmul(ps_m1[:], enc_b[:, k:k + 1],
                                 mp1e_s[:, ts(k, 256)],
                                 start=(k == 0), stop=False)
            for k in range(8):
                nc.tensor.matmul(ps_m1[:], ppad_b[:, k:k + 1],
                                 mp1p_s[:, ts(k, 256)],
                                 start=False, stop=(k == 7))
            m1acc = cpool.tile([1, 256], f32, tag="m1acc")
            nc.vector.tensor_tensor(out=m1acc[:], in0=ps_m1[:],
                                    in1=mp_b1_sb[:], op=ALU.add)

            # ================= K-PASS =================
            # paired transposed weights: slot t2 covers m-tiles 2*t2, 2*t2+1
            wpair = cpool.tile([128, 25 * 32], f8, tag="wpair")
            nc.vector.memset(wpair[:], 0.0)
            zacc = cpool.tile([8, 16], f32, tag="zacc")
            nc.vector.memset(zacc[:], 0.0)
            for mc in range(NCH):
                cw = 512 if mc < 12 else 128
                ps_s = pss.tile([8, 512], f32, tag="s")
                khv = khs[:].rearrange("p (j m) -> p j m", m=MPAD)
                for pj in range(4):
                    nc.tensor.matmul(
                        ps_s[:, 0:cw],
                        qkm[:, pj * 32:(pj + 1) * 32]
                        .rearrange("p (i h) -> p i h", i=2)[:, :, 0:8],
                        khv[:, 2 * pj:2 * pj + 2, mc * 512:mc * 512 + cw],
                        start=(pj == 0), stop=(pj == 3),
                        perf_mode=mybir.MatmulPerfMode.DoubleRow)
                w_c = cpool.tile([8, 512], f32, tag="w_c")
                if mc < 12:
                    nc.scalar.activation(w_c[:, 0:cw], ps_s[:, 0:cw], ACTF.Exp,
                                         bias=c_sb[:], scale=1.0 / S8,
                                         accum_out=zacc[:, mc:mc + 1])
                else:
                    nc.scalar.activation(w_c[:, 0:cw], ps_s[:, 0:cw], ACTF.Exp,
                                         bias=c_sb[:], scale=1.0 / S8)
                    nc.vector.memset(w_c[:, MVALID_TAIL:cw], 0.0)
                    nc.vector.tensor_reduce(out=zacc[:, mc:mc + 1],
                                            in_=w_c[:, 0:cw], axis=AX.X,
                                            op=ALU.add)
                if debug_taps and mc == 0:
                    dma(dbg["dbg_w0"].ap(), w_c[:])
                for ti in range(cw // 128):
                    gt = mc * 4 + ti
                    pt = pstr.tile([128, 8], f32, tag="tr")
                    nc.tensor.transpose(pt[:], w_c[0:8, ts(ti, 128)],
                                        ident[0:8, 0:8])
                    dst = (gt // 2) * 32 + (gt % 2) * 16
                    nc.vector.tensor_copy(wpair[:, dst:dst + 8], pt[:])

            # local Z per head
            zloc = cpool.tile([8, 1], f32, tag="zloc")
            nc.vector.tensor_reduce(out=zloc[:], in_=zacc[:, 0:NCH], axis=AX.X,
                                    op=ALU.add)
            if debug_taps:
                dma(dbg["dbg_z"].ap(), zacc[:])

            # ---- top-5 candidates (overlaps V-pass) ----
            rz8 = cpool.tile([8, 1], f32, tag="rz8")
            nc.vector.reciprocal(rz8[:], zloc[:])
            nc.vector.tensor_scalar_mul(rz8[:], rz8[:], 1.0 / (NH * NCN))
            zq = col_rep(rz8[:], "zq")
            awf = cpool.tile([128, 50 * 8], f32, tag="awf")
            nc.vector.tensor_tensor(
                out=awf[:].rearrange("p (a b) -> p a b", b=8),
                in0=wpair[:].rearrange("p (a g b) -> p a g b", g=2, b=8)[:, :, 0, :],
                in1=_b3(zq, 50), op=ALU.mult)
            attnw = cpool.tile([128, 50], f32, tag="attnw")
            nc.vector.tensor_reduce(out=attnw[:],
                                    in_=awf[:].rearrange("p (a b) -> p a b", b=8),
                                    axis=AX.X, op=ALU.add)
            cand1 = cpool.tile([128, 8], f32, tag="cand1")
            nc.vector.max(out=cand1[:], in_=attnw[:])
            ptc1 = pstr.tile([8, 128], f32, tag="tr")
            nc.tensor.transpose(ptc1[:], cand1[:], ident[:, :])
            cd2 = cpool.tile([8, 128], f32, tag="cd2")
            nc.vector.tensor_copy(cd2[:], ptc1[:])
            cand2 = cpool.tile([8, 8], f32, tag="cand2")
            nc.vector.max(out=cand2[:], in_=cd2[:])
            d_c64 = dpool.tile([64], f32, tag="d_c64")
            dma(d_c64[:].rearrange("(p b) -> p b", b=8), cand2[:])
            c64 = cpool.tile([1, 64], f32, tag="c64")
            dma(c64[:], d_c64[:].rearrange("(o b) -> o b", o=1))
            top8 = cpool.tile([1, 8], f32, tag="top8")
            nc.vector.max(out=top8[:], in_=c64[:])
            slots = cpool.tile([1, 5 * NCN], f32, tag="slots")
            for i in range(NCN):
                nc.vector.tensor_copy(slots[:, i * 5:(i + 1) * 5], top8[:, 0:5])
            nc.vector.tensor_mul(slots[:], slots[:], slot_sb[:])

            # ================= V-PASS =================
            ps_u = []
            for nb in range(4):
                ps_unb = psu.tile([8, 512], f32, tag=f"u{nb}", name=f"ps_u{nb}")
                ps_u.append(ps_unb)
            for cd in range(NCH):
                ntile = 4 if cd < 12 else 1
                rows = 512 if cd < 12 else 128
                vt = vpool.tile([128, 4 * HID], f8, tag="vt")
                dma(vt[:, 0:ntile * HID].rearrange("p (mc d) -> p mc d", d=HID),
                    vh.ap()[cd * 512: cd * 512 + rows, :]
                    .rearrange("(mc p) d -> p mc d", p=128))
                vtv = vt[:].rearrange("p (mc d) -> p mc d", d=HID)
                if cd < 12:
                    for t2l in range(2):
                        t2 = cd * 2 + t2l
                        for nb in range(4):
                            nc.tensor.matmul(
                                ps_u[nb][:],
                                wpair[:, t2 * 32:(t2 + 1) * 32]
                                .rearrange("p (i h) -> p i h", i=2)[:, :, 0:8],
                                vtv[:, 2 * t2l:2 * t2l + 2,
                                    nb * 512:nb * 512 + 512],
                                start=(t2 == 0), stop=False,
                                perf_mode=mybir.MatmulPerfMode.DoubleRow)
                else:
                    for nb in range(4):
                        nc.tensor.matmul(ps_u[nb][:],
                                         wpair[:, 24 * 32:24 * 32 + 8],
                                         vt[:, nb * 512:nb * 512 + 512],
                                         start=False, stop=True)

            # ---- u AllReduce payload: [8, 2048 u | 1 Z | 40 slots | 7 pad]
            UW = 3896
            d_u_i = dpool.tile([8, UW], bf16, tag="d_u_i")
            d_u_o = dpool.tile([8, UW], bf16, tag="d_u_o")
            u_s = cpool.tile([8, HID], bf16, tag="u_s")
            for nb in range(4):
                nc.vector.tensor_copy(u_s[:, ts(nb, 512)], ps_u[nb][:])
            dma(d_u_i[:].rearrange("h w -> (h w)")[ds(1792, 8 * (UW - 256))]
                .rearrange("(h d) -> h d", d=UW - 256)[:, 0:HID],
                u_s[:])
            stg = cpool.tile([8, 48], bf16, tag="stg")
            nc.vector.memset(stg[:], 0.0)
            nc.vector.tensor_copy(stg[:, 0:1], zloc[:])
            nc.vector.tensor_copy(stg[0:1, 1:41], slots[:])
            dma(d_u_i[:, 3840:3888], stg[:])
            nc.gpsimd.collective_compute("AllReduce", ALU.add, replica_groups=RG,
                                         ins=[d_u_i.opt()], outs=[d_u_o.opt()])

            # ---- post-AR: Z, top5, ctx extraction ----
            G = cpool.tile([8, 48], bf16, tag="G")
            dma(G[:], d_u_o[:, 3840:3888])
            zg = cpool.tile([8, 1], f32, tag="zg")
            nc.vector.reciprocal(zg[:], G[:, 0:1])
            top40 = cpool.tile([1, 5 * NCN], f32, tag="top40")
            nc.vector.tensor_copy(top40[:], G[0:1, 1:41])
            top8f = cpool.tile([1, 8], f32, tag="top8f")
            nc.vector.max(out=top8f[:], in_=top40[:])

            ctxm_b = cpool.tile([8, 256], bf16, tag="ctxm_b")
            dma(ctxm_b[:], d_u_o[:, 1792:2048])
            ctxm = cpool.tile([8, 256], f32, tag="ctxm")
            nc.vector.tensor_scalar(out=ctxm[:], in0=ctxm_b[:], scalar1=zg[:],
                                    scalar2=None, op0=ALU.mult)
            if debug_taps:
                dma(dbg["dbg_u"].ap(),
                    d_u_o[:].rearrange("h w -> (h w)")[ds(1792, 8 * (UW - 256))]
                    .rearrange("(h d) -> h d", d=UW - 256)[:, 0:HID])
            ps_cr = psx.tile([1, 256], f32, tag="px")
            nc.tensor.matmul(ps_cr[:], oh8_sb[:], ctxm[:], start=True, stop=True)
            ctx_row = cpool.tile([1, 256], f32, tag="ctx_row")
            nc.vector.tensor_copy(ctx_row[:], ps_cr[:])
            ctx_sb = row_T(ctx_row, 2, "ctx_sb")
            # full ctx in p-major layout for the folded m1-attended stage:
            # col k holds ctx[k*128 : (k+1)*128] = ctxm[k//2, (k%2)*128 : +128]
            ctx_pm = cpool.tile([128, 16], bf16, tag="ctx_pm")
            ctxT = cpool.tile([128, 16], f32, tag="ctxT")
            for j in range(2):
                ptc = pstr.tile([128, 8], f32, tag="tr")
                nc.tensor.transpose(ptc[:], ctxm[0:8, j * 128:(j + 1) * 128],
                                    ident[0:8, 0:8])
                nc.vector.tensor_copy(ctxT[:, ts(j, 8)], ptc[:])
            for k in range(16):
                nc.vector.tensor_copy(ctx_pm[:, k:k + 1],
                                      ctxT[:, (k % 2) * 8 + k // 2:
                                           (k % 2) * 8 + k // 2 + 1])
            if debug_taps:
                dma(dbg["dbg_ctx"].ap(), ctx_sb[:])

            # attended partial = wo[:, b2] @ ctx_c   (row orientation)
            att_p = cpool.tile([1, HID], f32, tag="rowst", name="att_p")
            for nb in range(4):
                ps_at = pss.tile([1, 512], f32, tag="s")
                for kc in range(2):
                    nc.tensor.matmul(ps_at[:], ctx_sb[:, kc:kc + 1],
                                     woS[:, kc * HID + nb * 512:
                                         kc * HID + nb * 512 + 512],
                                     start=(kc == 0), stop=(kc == 1))
                nc.vector.tensor_copy(att_p[:, ts(nb, 512)], ps_at[:])
            d_a_i = dpool.tile([1, HID], f32, tag="d_a_i")
            d_a_o = dpool.tile([1, HID], f32, tag="d_a_o")
            dma(d_a_i[:], att_p[:])
            # m1 stage B from folded (mp1a @ wo) @ ctx — independent of att-AR
            ps_m1b = psx.tile([1, 256], f32, tag="px")
            for k in range(16):
                nc.tensor.matmul(ps_m1b[:], ctx_pm[:, k:k + 1],
                                 mp1a_s[:, ts(k, 256)],
                                 start=(k == 0), stop=(k == 15))
            m1_row = cpool.tile([1, 256], f32, tag="m1_row")
            nc.vector.tensor_add(m1_row[:], ps_m1b[:], m1acc[:])
            nc.vector.tensor_scalar_max(m1_row[:], m1_row[:], 0.0)
            nc.vector.tensor_mul(m1_row[:], m1_row[:], bn2sc_sb[:])
            nc.vector.tensor_add(m1_row[:], m1_row[:], bn2sh_sb[:])
            if debug_taps:
                dma(dbg["dbg_m1"].ap(), m1_row[:])
            m1_sb = row_T(m1_row, 2, "m1_sb")

            # m2 partial = mp_w2[:, b2] @ m1_c   (row orientation)
            m2_pf = cpool.tile([1, HID], f32, tag="rowst", name="m2_pf")
            m2_p = m2_pf[0:1, 0:H2]
            for nb in range(2):
                ps_m2 = pss.tile([1, 512], f32, tag="s")
                for kc in range(2):
                    nc.tensor.matmul(ps_m2[:], m1_sb[:, kc:kc + 1],
                                     mp_w2s[:, kc * H2 + nb * 512:
                                            kc * H2 + nb * 512 + 512],
                                     start=(kc == 0), stop=(kc == 1))
                nc.vector.tensor_copy(m2_p[:, ts(nb, 512)], ps_m2[:])
            d_m2_i = dpool.tile([1, H2], f32, tag="d_m2_i")
            d_m2_o = dpool.tile([1, H2], f32, tag="d_m2_o")
            dma(d_m2_i[:], m2_p)
            nc.gpsimd.collective_compute("AllReduce", ALU.add, replica_groups=RG,
                                         ins=[d_m2_i.opt()], outs=[d_m2_o.opt()])
            nc.gpsimd.collective_compute("AllReduce", ALU.add, replica_groups=RG,
                                         ins=[d_a_i.opt()], outs=[d_a_o.opt()])
            m2_sb = cpool.tile([128, 8], f32, tag="m2_sb")
            dma(m2_sb[:], d_m2_o[:].rearrange("o (k p) -> (o p) k", p=128))
            nc.vector.tensor_add(m2_sb[:], m2_sb[:], mp_b2_sb[:])
            nc.vector.tensor_scalar_max(m2_sb[:], m2_sb[:], 0.0)
            m2_b = cpool.tile([128, 8], bf16, tag="m2_b")
            nc.vector.tensor_copy(m2_b[:], m2_sb[:])

            # meta = mp_w3 @ m2 + b3
            ps_mt = psx.tile([4, 1], f32, tag="px")
            for k in range(8):
                nc.tensor.matmul(ps_mt[:], mp_w3s[:, ts(k, 4)],
                                 m2_b[:, k:k + 1],
                                 start=(k == 0), stop=(k == 7))
            meta_sb = cpool.tile([4, 1], f32, tag="meta_sb")
            nc.vector.tensor_add(meta_sb[:], ps_mt[:], mp_b3_sb[:])
            ptmt = pstr.tile([1, 4], f32, tag="tr")
            nc.tensor.transpose(ptmt[:], meta_sb[:], ident[0:4, 0:4])
            metaT = cpool.tile([1, 4], f32, tag="metaT")
            nc.vector.tensor_copy(metaT[:], ptmt[:])
            nmax = cpool.tile([1, 1], f32, tag="nmax")
            nc.vector.tensor_reduce(out=nmax[:], in_=metaT[:, 0:3], axis=AX.X,
                                    op=ALU.max)
            nc.vector.tensor_scalar_mul(nmax[:], nmax[:], -1.0)
            e3 = cpool.tile([1, 3], f32, tag="e3")
            nc.scalar.activation(e3[:], metaT[:, 0:3], ACTF.Exp, bias=nmax[:])
            s3 = cpool.tile([1, 1], f32, tag="s3")
            nc.vector.tensor_reduce(out=s3[:], in_=e3[:], axis=AX.X, op=ALU.add)
            nc.vector.reciprocal(s3[:], s3[:])
            regime = cpool.tile([1, 3], f32, tag="regime")
            nc.vector.tensor_scalar(out=regime[:], in0=e3[:], scalar1=s3[:],
                                    scalar2=None, op0=ALU.mult)
            crisis = cpool.tile([1, 1], f32, tag="crisis")
            nc.scalar.activation(crisis[:], metaT[:, 3:4], ACTF.Sigmoid)

            att_f = cpool.tile([128, 16], f32, tag="att_f")
            dma(att_f[:], d_a_o[:].rearrange("o (k p) -> (o p) k", p=128))
            nc.vector.tensor_add(att_f[:], att_f[:], bob_sb[:])
            dma(out.ap()[2058:4106].rearrange("(b p) -> p b", p=128), att_f[:])

            # ---- output assembly ----
            dma(out.ap()[0:3].rearrange("(o b) -> o b", o=1), regime[:])
            dma(out.ap()[3:4].rearrange("(o b) -> o b", o=1), crisis[:])
            dma(out.ap()[4:5].rearrange("(o b) -> o b", o=1), conf[:])
            dma(out.ap()[5:10].rearrange("(o b) -> o b", o=1), top8f[:, 0:5])

    nc.compile()
    return nc


_NC_CACHE = {}


def _get_nc():
    if "nc" not in _NC_CACHE:
        _NC_CACHE["nc"] = build_nc()
    return _NC_CACHE["nc"]


def _bm(x, nb):
    """vector [nb*128] -> b-major [128, nb] (col b = x[b*128:(b+1)*128])."""
    return np.ascontiguousarray(np.asarray(x, np.float32).reshape(nb, 128).T)


def _bf(x):
    return np.ascontiguousarray(np.asarray(x)).astype(ml_dtypes.bfloat16)


def _f8(x):
    return np.ascontiguousarray(np.asarray(x)).astype(ml_dtypes.float8_e4m3)


def shard_inputs(i):
    g = {k: np.asarray(v, np.float32) for k, v in i.items()}
    # host folds
    kh = g["memory_keys"] @ g["wk"].T                       # [MEM, H2]
    vhf = g["memory_values"] @ g["wv"].T                    # [MEM, HID]
    WQ2 = (g["wq"] @ g["qe_w2"]) * ISCALE                   # [H2, HID]
    qbias_full = (g["wq"] @ g["qe_b2"] + g["bq"]) * ISCALE  # [H2]
    bob_full = g["wo"] @ g["bv"] + g["bo"]                  # [HID]
    bn1_scale = g["bn1_g"] / np.sqrt(g["bn1_v"] + EPS)
    bn1_shift = g["bn1_b"] - g["bn1_m"] * bn1_scale
    bn2_scale = g["bn2_g"] / np.sqrt(g["bn2_v"] + EPS)
    bn2_shift = g["bn2_b"] - g["bn2_m"] * bn2_scale

    in_maps = []
    for c in range(NCN):
        b2 = slice(c * 256, (c + 1) * 256)
        khp = np.zeros((MPAD, H2), np.float32)
        khp[0:MSH] = kh[c * MSH:(c + 1) * MSH]
        vhp = np.zeros((MPAD, HID), np.float32)
        vhp[0:MSH] = vhf[c * MSH:(c + 1) * MSH]
        oh = np.zeros((8, 1), np.float32); oh[c, 0] = 1.0
        sm = np.zeros((1, 40), np.float32); sm[0, c * 5:(c + 1) * 5] = 1.0
        m = {
            "obs": _bf(_bm(g["observation"], 32)),
            "w1T": _bf(g["ce_w1"][b2].T),
            "ce_b1r": g["ce_b1"][b2].reshape(1, 256),
            "bn1_sc": bn1_scale[b2].reshape(1, 256),
            "bn1_sh": bn1_shift[b2].reshape(1, 256),
            "ce_w2T": _bf(g["ce_w2"][:, b2].T),
            "encbr": (g["ce_b2"] if c == 0
                      else np.zeros(HID, np.float32)).reshape(1, HID),
            "qe_w1T": _bf(g["qe_w1"][b2].T),
            "qe_b1r": g["qe_b1"][b2].reshape(1, 256),
            "wq2T": _bf(WQ2[:, b2].T),
            "qbias": (_bm(qbias_full, 8) if c == 0
                      else np.zeros((128, 8), np.float32)),
            "bk8": _bm(g["bk"], 8),
            "khT": _f8(khp.T),
            "vh": _f8(vhp),
            "woT": _bf(g["wo"][:, b2].T),
            "bob": _bm(bob_full, 16),
            "protos": np.ascontiguousarray(g["prototypes"]),
            "mp1eT": _bf(g["mp_w1"][b2, 0:HID].T),
            "mp1aT": _bf((g["mp_w1"][b2, HID:2 * HID] @ g["wo"]).T),
            "mp1pT": _bf(g["mp_w1"][b2, 2 * HID:2 * HID + H2].T),
            "mp_b1r": (g["mp_b1"][b2]
                       + g["mp_w1"][b2, HID:2 * HID] @ bob_full
                       ).reshape(1, 256),
            "bn2_sc": bn2_scale[b2].reshape(1, 256),
            "bn2_sh": bn2_shift[b2].reshape(1, 256),
            "mp_w2T": _bf(g["mp_w2"][:, b2].T),
            "mp_b2_8": _bm(g["mp_b2"], 8),
            "mp_w3T": _bf(g["mp_w3"].T),
            "mp_b3": np.asarray(g["mp_b3"], np.float32).reshape(4, 1).copy(),
            "oh8": oh,
            "slot_mask": sm,
        }
        in_maps.append(m)
    return in_maps


def kernel(**inputs):
    nc = _get_nc()
    in_maps = shard_inputs(inputs)
    res = bass_utils.run_bass_kernel_spmd(nc, in_maps, core_ids=list(range(NCN)))
    return np.asarray(res.results[0]["out"], np.float32)
